# revision 1
# baseline (speedup 1.0000x reference)
"""GAT (2-layer, PyG-style) forward on 8 TRN2 NeuronCores.

Sharding: dst-node blocks across cores (host-permuted for per-block edge-count
balance); per-core edge lists routed by dst block on host; per-edge payload
gathered from a replicated node table via one batched indirect DMA per block;
al_dst broadcast via a second tiny indirect gather on the dst indices; segment
softmax + weighted sum via selection-matrix matmuls in PSUM; work spread across
DVE/Act/Pool engines."""
import sys
if '/opt/trn_rl_repo' not in sys.path:
    sys.path.insert(0, '/opt/trn_rl_repo')
import json
import numpy as np
import ml_dtypes

import concourse.bass as bass
import concourse.mybir as mybir
import concourse.tile as tile
from concourse import library_config

bf16 = ml_dtypes.bfloat16
F32 = mybir.dt.float32
BF16 = mybir.dt.bfloat16
I32 = mybir.dt.int32
ALU = mybir.AluOpType
ACTF = mybir.ActivationFunctionType


def mkap(ap, dims, elem_offset=0):
    """AP with explicit [step, count] free dims (elements) after the partition dim."""
    return bass.AP(ap.tensor, ap.offset + elem_offset,
                   [list(ap.ap[0])] + [list(d) for d in dims])


def build_gat_layer(N, FIN, H, C, T_blk, n_blocks, core_rows, mode, with_bias=True,
                    tile_rows=None):
    """mode: 'elu' (layer 1) or 'mean_lsm' (layer 2)."""
    FOUT = H * C
    TCOLS = FOUT + 2 * H          # table row: [h | al_src | al_dst]
    GCOLS = FOUT + H              # gathered per edge: [h | al_src]
    NT = n_blocks * T_blk
    KCH = FIN // 128
    last_blk_rows = core_rows - (n_blocks - 1) * 128

    nc = bass.Bass("TRN2", target_bir_lowering=False, debug=False, num_devices=8)

    xT = nc.dram_tensor("xT", [128, KCH, N], BF16, kind="ExternalInput")
    wcat = nc.dram_tensor("wcat", [128, KCH, TCOLS], BF16, kind="ExternalInput")
    bcat_in = nc.dram_tensor("bcat", [128, TCOLS], BF16, kind="ExternalInput")
    iota_in = nc.dram_tensor("iota", [128, 128], BF16, kind="ExternalInput")
    ones_in = nc.dram_tensor("ones", [128, 128], BF16, kind="ExternalInput")
    srcidx_in = nc.dram_tensor("srcidx", [128, NT], I32, kind="ExternalInput")
    dstloc_in = nc.dram_tensor("dstloc", [128, NT], BF16, kind="ExternalInput")
    dstlocT_in = nc.dram_tensor("dstlocT", [NT, 128], BF16, kind="ExternalInput")
    iotac_in = nc.dram_tensor("iotac", [128, 1], BF16, kind="ExternalInput")
    adrow_in = nc.dram_tensor("adrow", [128, n_blocks], I32, kind="ExternalInput")
    if mode == "elu":
        out_d = nc.dram_tensor("out", [core_rows, FOUT], BF16, kind="ExternalOutput")
        OCOLS = FOUT
        ODT = BF16
    else:
        out_d = nc.dram_tensor("out", [core_rows, C], F32, kind="ExternalOutput")
        OCOLS = C
        ODT = F32
    table = nc.dram_tensor("table", [N, TCOLS], BF16)

    ST = 16                      # node tiles per staging buffer / table-write DMA
    CH = ST * 128                # xT chunk columns
    n_ch = (N + CH - 1) // CH

    with tile.TileContext(nc) as tc:
        with (
            tc.tile_pool(name="const", bufs=1) as kpool,
            tc.tile_pool(name="xchunk", bufs=3) as xpool,
            tc.tile_pool(name="stage", bufs=3) as stpool,
            tc.tile_pool(name="dpsum", bufs=2, space="PSUM") as dppool,
            tc.tile_pool(name="g", bufs=5) as gpool,
            tc.tile_pool(name="ad", bufs=4) as adpool,
            tc.tile_pool(name="s01", bufs=3) as spool,
            tc.tile_pool(name="ee", bufs=4) as eepool,
            tc.tile_pool(name="mp", bufs=3) as mppool,
            tc.tile_pool(name="upsum", bufs=2, space="PSUM") as uppool,
            tc.tile_pool(name="epi", bufs=3) as epool,
            tc.tile_pool(name="oacc", bufs=1) as opool,
        ):
            # ---- constants ----
            wcat_sb = kpool.tile([128, KCH * TCOLS], BF16)
            nc.sync.dma_start(out=wcat_sb[:], in_=wcat[:].rearrange("p k c -> p (k c)"))
            bcat_sb = kpool.tile([128, TCOLS], BF16)
            nc.sync.dma_start(out=bcat_sb[:], in_=bcat_in[:])
            iota_sb = kpool.tile([128, 128], BF16)
            nc.sync.dma_start(out=iota_sb[:], in_=iota_in[:])
            ones_sb = kpool.tile([128, 128], BF16)
            nc.sync.dma_start(out=ones_sb[:], in_=ones_in[:])
            srcidx_sb = kpool.tile([128, NT], I32)
            nc.sync.dma_start(out=srcidx_sb[:], in_=srcidx_in[:])
            iotac_sb = kpool.tile([128, 1], BF16)
            nc.sync.dma_start(out=iotac_sb[:], in_=iotac_in[:])
            adrow_sb = kpool.tile([128, n_blocks], I32)
            nc.sync.dma_start(out=adrow_sb[:], in_=adrow_in[:])
            dstloc_sb = kpool.tile([128, NT], BF16)
            nc.sync.dma_start(out=dstloc_sb[:], in_=dstloc_in[:])
            neg1 = kpool.tile([128, 1], F32)
            nc.vector.memset(neg1[:], -1.0)

            obuf = opool.tile([128, n_blocks * OCOLS], ODT)

            # ---- dense phase: table[N, TCOLS] = [x @ Wcat + b'] ----
            # first chunks are small so the edge-phase gathers unlock early
            chunks = []
            c0 = 0
            for w in [512, 512, 1024]:
                chunks.append((c0, w)); c0 += w
            while c0 < N:
                w = min(CH, N - c0)
                chunks.append((c0, w)); c0 += w
            for ci, (c0, ccols) in enumerate(chunks):
                nt_ch = (ccols + 127) // 128
                xc = xpool.tile([128, KCH * CH], BF16, tag="xc")
                nc.sync.dma_start(
                    out=mkap(xc[:], [[CH, KCH], [1, ccols]]),
                    in_=xT[:, :, c0:c0 + ccols],
                )
                st = stpool.tile([128, ST * TCOLS], BF16, tag="st")
                full = ccols == CH
                if full:
                    # pairs of node tiles share one 2-bank PSUM tile; one copy per pair
                    for pr in range(ST // 2):
                        psum = dppool.tile([128, 1024], F32, tag="dp")
                        for sub in range(2):
                            tl = pr * 2 + sub
                            col = tl * 128
                            for k in range(KCH):
                                nc.tensor.matmul(
                                    psum[:, sub * 512: sub * 512 + TCOLS],
                                    lhsT=xc[:, k * CH + col: k * CH + col + 128],
                                    rhs=wcat_sb[:, k * TCOLS:(k + 1) * TCOLS],
                                    start=(k == 0),
                                    stop=(not with_bias and k == KCH - 1),
                                )
                            if with_bias:
                                nc.tensor.matmul(
                                    psum[:, sub * 512: sub * 512 + TCOLS],
                                    lhsT=ones_sb[:],
                                    rhs=bcat_sb[:],
                                    start=False, stop=True,
                                )
                        dst_ap = mkap(st[:], [[TCOLS, 2], [1, TCOLS]],
                                      elem_offset=pr * 2 * TCOLS)
                        src_ap = mkap(psum[:], [[512, 2], [1, TCOLS]])
                        eng = (ci * (ST // 2) + pr) % 2
                        if eng == 0:
                            nc.scalar.copy(out=dst_ap, in_=src_ap)
                        else:
                            nc.vector.tensor_copy(out=dst_ap, in_=src_ap)
                else:
                    for tl in range(nt_ch):
                        rows = min(128, ccols - tl * 128)
                        col = tl * 128
                        psum = dppool.tile([128, 1024], F32, tag="dp")
                        for k in range(KCH):
                            nc.tensor.matmul(
                                psum[:rows, 0:TCOLS],
                                lhsT=xc[:, k * CH + col: k * CH + col + rows],
                                rhs=wcat_sb[:, k * TCOLS:(k + 1) * TCOLS],
                                start=(k == 0),
                                stop=(not with_bias and k == KCH - 1),
                            )
                        if with_bias:
                            nc.tensor.matmul(
                                psum[:rows, 0:TCOLS],
                                lhsT=ones_sb[:, 0:rows],
                                rhs=bcat_sb[:],
                                start=False, stop=True,
                            )
                        nc.scalar.copy(
                            out=st[:rows, tl * TCOLS:(tl + 1) * TCOLS],
                            in_=psum[:rows, 0:TCOLS])
                # flush: full 128-row tiles in one strided DMA, partial tail separately
                n_full = ccols // 128
                if n_full:
                    nc.sync.dma_start(
                        out=table[c0:c0 + n_full * 128, :]
                        .rearrange("(a p) c -> p a c", p=128),
                        in_=mkap(st[:], [[TCOLS, n_full], [1, TCOLS]]),
                    )
                rem = ccols - n_full * 128
                if rem:
                    nc.sync.dma_start(
                        out=table[c0 + n_full * 128: c0 + ccols, :],
                        in_=st[:rem, n_full * TCOLS:(n_full + 1) * TCOLS],
                    )

            # ---- edge phase (software-pipelined epilogue: epi(b) after front(b+1)) ----
            table_flat = bass.AP(table[:].tensor, 0,
                                 [[N * TCOLS, 1], [1, N * TCOLS]])
            up_tiles = {}

            def edge_front(b):
                bT = b * T_blk
                # per-edge payload gathers: one indirect DMA per 128-edge tile
                g_blk = gpool.tile([128, T_blk * GCOLS], BF16, tag="g")
                for t in range(T_blk):
                    if tile_rows is None:
                        src_ap = table_flat
                    else:
                        L = int(tile_rows[b][t]) * TCOLS
                        src_ap = bass.AP(table[:].tensor, 0, [[L, 1], [1, L]])
                    nc.gpsimd.indirect_dma_start(
                        out=g_blk[:, t * GCOLS:(t + 1) * GCOLS],
                        out_offset=None,
                        in_=src_ap,
                        in_offset=bass.IndirectOffsetOnAxis(
                            ap=srcidx_sb[:, bT + t:bT + t + 1], axis=1),
                    )
                # s01[j, (t,d)] = (dstloc[j, bT+t] == d)
                s01 = spool.tile([128, T_blk * 128], BF16, tag="s01")
                nc.vector.tensor_tensor(
                    out=s01[:],
                    in0=mkap(dstloc_sb[:], [[1, T_blk], [0, 128]], elem_offset=bT),
                    in1=mkap(iota_sb[:], [[0, T_blk], [1, 128]]),
                    op=ALU.is_equal,
                )
                # al_dst broadcast to edges: S01T[d,(t,j)] from partition-bcast
                # dstlocT, then T small matmuls vs this block's al_dst rows
                rep = spool.tile([128, T_blk * 128], BF16, tag="rep")
                nc.sync.dma_start(
                    out=rep[:],
                    in_=bass.AP(dstlocT_in[:].tensor, bT * 128,
                                [[0, 128], [1, T_blk * 128]]),
                )
                s01T = spool.tile([128, T_blk * 128], BF16, tag="s01T")
                nc.vector.tensor_tensor(
                    out=s01T[:],
                    in0=rep[:],
                    in1=mkap(iotac_sb[:], [[0, T_blk], [0, 128]]),
                    op=ALU.is_equal,
                )
                adrow = adpool.tile([128, TCOLS], BF16, tag="adrow")
                nc.gpsimd.indirect_dma_start(
                    out=adrow[:], out_offset=None, in_=table_flat,
                    in_offset=bass.IndirectOffsetOnAxis(ap=adrow_sb[:, b:b + 1],
                                                        axis=1),
                )
                adp = uppool.tile([128, T_blk * H], F32, tag="adp")
                for t in range(T_blk):
                    nc.tensor.matmul(
                        adp[:, t * H:(t + 1) * H],
                        lhsT=s01T[:, t * 128:(t + 1) * 128],
                        rhs=adrow[:, FOUT + H:FOUT + 2 * H],
                        start=True, stop=True,
                    )
                # self-loop: e = lrelu(als+ald) of own row; initializes PSUM
                ssf = eepool.tile([128, H], F32, tag="ssf")
                nc.vector.tensor_tensor(
                    out=ssf[:], in0=adrow[:, FOUT:FOUT + H],
                    in1=adrow[:, FOUT + H:FOUT + 2 * H], op=ALU.add)
                slr = eepool.tile([128, H], F32, tag="slr")
                nc.vector.scalar_tensor_tensor(
                    out=slr[:], in0=ssf[:], scalar=0.2, in1=ssf[:],
                    op0=ALU.mult, op1=ALU.max)
                see = eepool.tile([128, H], F32, tag="see")
                nc.scalar.activation(see[:], slr[:], ACTF.Exp)
                # s = as + ad ; lrelu ; ee = exp
                s_f = eepool.tile([128, T_blk * H], F32, tag="sf")
                nc.vector.tensor_tensor(
                    out=s_f[:],
                    in0=mkap(g_blk[:], [[GCOLS, T_blk], [1, H]], elem_offset=FOUT),
                    in1=adp[:],
                    op=ALU.add,
                )
                lr = eepool.tile([128, T_blk * H], F32, tag="lr")
                nc.vector.scalar_tensor_tensor(
                    out=lr[:], in0=s_f[:], scalar=0.2, in1=s_f[:],
                    op0=ALU.mult, op1=ALU.max)
                ee = eepool.tile([128, T_blk * H], BF16, tag="ee")
                nc.scalar.activation(ee[:], lr[:], ACTF.Exp)
                # m' = ee*h (bf16)
                mp = mppool.tile([128, T_blk * FOUT], BF16, tag="mp")
                nc.vector.tensor_tensor(
                    out=mp[:],
                    in0=mkap(g_blk[:], [[GCOLS, T_blk], [1, FOUT]]),
                    in1=mkap(ee[:], [[H, T_blk], [1, H], [0, C]]),
                    op=ALU.mult,
                )
                # segment-sum matmuls into PSUM, initialized with the
                # self-loop contribution (ee_self * h_self | ee_self)
                up = uppool.tile([128, FOUT + H], F32, tag="up")
                up_tiles[b] = up
                nc.vector.tensor_tensor(
                    out=up[:, 0:FOUT],
                    in0=adrow[:, 0:FOUT],
                    in1=mkap(see[:], [[1, H], [0, C]]),
                    op=ALU.mult)
                nc.vector.tensor_copy(out=up[:, FOUT:FOUT + H], in_=see[:])
                for t in range(T_blk):
                    lhs = s01[:, t * 128:(t + 1) * 128]
                    nc.tensor.matmul(
                        up[:, 0:FOUT],
                        lhsT=lhs,
                        rhs=mp[:, t * FOUT:(t + 1) * FOUT],
                        start=False,
                        stop=(t == T_blk - 1),
                    )
                    nc.tensor.matmul(
                        up[:, FOUT:FOUT + H],
                        lhsT=lhs,
                        rhs=ee[:, t * H:(t + 1) * H],
                        start=False,
                        stop=(t == T_blk - 1),
                    )
            def edge_epi(b):
                brows = 128 if b < n_blocks - 1 else last_blk_rows
                up = up_tiles.pop(b)
                rec = epool.tile([128, H], F32, tag="rec")
                nc.vector.reciprocal(out=rec[:brows], in_=up[:brows, FOUT:FOUT + H])
                u = epool.tile([128, FOUT], F32, tag="u")
                for h in range(H):
                    nc.scalar.mul(u[:brows, h * C:(h + 1) * C],
                                  up[:brows, h * C:(h + 1) * C],
                                  rec[:brows, h:h + 1])
                if mode == "elu":
                    nr = epool.tile([128, FOUT], F32, tag="nr")
                    nc.scalar.activation(nr[:brows], u[:brows], ACTF.Relu, scale=-1.0)
                    ex = epool.tile([128, FOUT], F32, tag="ex")
                    nc.scalar.activation(ex[:brows], nr[:brows], ACTF.Exp, scale=-1.0)
                    sm = epool.tile([128, FOUT], F32, tag="sm")
                    nc.vector.scalar_tensor_tensor(
                        out=sm[:brows], in0=u[:brows], scalar=0.0, in1=ex[:brows],
                        op0=ALU.max, op1=ALU.add)
                    nc.scalar.activation(
                        obuf[:brows, b * OCOLS:(b + 1) * OCOLS], sm[:brows],
                        ACTF.Identity, bias=neg1[:brows])
                else:
                    m1 = epool.tile([128, FOUT // 2], F32, tag="m1")
                    nc.vector.tensor_tensor(out=m1[:brows], in0=u[:brows, 0:FOUT // 2],
                                            in1=u[:brows, FOUT // 2:FOUT], op=ALU.add)
                    m2 = epool.tile([128, FOUT // 4], F32, tag="m2")
                    nc.vector.tensor_tensor(out=m2[:brows], in0=m1[:brows, 0:FOUT // 4],
                                            in1=m1[:brows, FOUT // 4:FOUT // 2],
                                            op=ALU.add)
                    zb = epool.tile([128, C], F32, tag="zb")
                    nc.vector.tensor_tensor(out=zb[:brows], in0=m2[:brows, 0:C],
                                            in1=m2[:brows, C:2 * C], op=ALU.add)
                    zbb = epool.tile([128, C], F32, tag="zbb")
                    nc.vector.tensor_scalar_mul(out=zbb[:brows], in0=zb[:brows],
                                                scalar1=1.0 / H)
                    mxr = epool.tile([128, 1], F32, tag="mxr")
                    nc.vector.reduce_max(out=mxr[:brows], in_=zbb[:brows],
                                         axis=mybir.AxisListType.X)
                    nmx = epool.tile([128, 1], F32, tag="nmx")
                    nc.vector.tensor_scalar_mul(out=nmx[:brows], in0=mxr[:brows],
                                                scalar1=-1.0)
                    exs = epool.tile([128, C], F32, tag="exs")
                    sms = epool.tile([128, 1], F32, tag="sms")
                    nc.scalar.activation(exs[:brows], zbb[:brows], ACTF.Exp,
                                         bias=nmx[:brows], accum_out=sms[:brows])
                    lg = epool.tile([128, 1], F32, tag="lg")
                    nc.scalar.activation(lg[:brows], sms[:brows], ACTF.Ln)
                    nb = epool.tile([128, 1], F32, tag="nb")
                    nc.vector.tensor_tensor(out=nb[:brows], in0=nmx[:brows],
                                            in1=lg[:brows], op=ALU.subtract)
                    nc.scalar.activation(
                        obuf[:brows, b * OCOLS:(b + 1) * OCOLS], zbb[:brows],
                        ACTF.Identity, bias=nb[:brows])

            for b in range(n_blocks):
                edge_front(b)
                if b >= 1:
                    edge_epi(b - 1)
            edge_epi(n_blocks - 1)

            # ---- output flush: full blocks in one DMA, tail separately ----
            nfb = n_blocks - 1
            nc.sync.dma_start(
                out=out_d[0:nfb * 128, :].rearrange("(a p) c -> p a c", p=128),
                in_=mkap(obuf[:], [[OCOLS, nfb], [1, OCOLS]]),
            )
            nc.sync.dma_start(
                out=out_d[nfb * 128:core_rows, :],
                in_=obuf[:last_blk_rows, nfb * OCOLS:(nfb + 1) * OCOLS],
            )
    return nc


# ---------------- host side ----------------

def fold_weights(W, a_src, a_dst, H, C):
    """Wcat [FIN, H*C + 2H] f32: [W.T | Wa_src | Wa_dst]."""
    WT = np.asarray(W, np.float32).T.copy()           # [FIN, H*C]
    FIN = WT.shape[0]
    W3 = WT.reshape(FIN, H, C)
    Wa_s = np.einsum('fhc,hc->fh', W3, np.asarray(a_src, np.float32))
    Wa_d = np.einsum('fhc,hc->fh', W3, np.asarray(a_dst, np.float32))
    return np.concatenate([WT, Wa_s, Wa_d], axis=1)


def pack_kdim(M):
    """[FIN, COLS] -> [128, KCH, COLS]: row k*128+p -> [p, k]."""
    FIN, COLS = M.shape
    KCH = FIN // 128
    return np.ascontiguousarray(M.reshape(KCH, 128, COLS).transpose(1, 0, 2))


def route_edges(src, dst, n_cores, core_rows, n_nodes):
    """Balanced dst-node placement + per-core edge routing.

    Returns (T_blk, n_blocks, perm_pos[n_nodes], per-core dict of
    srcidx/dstidx [128,NT] i32 and dstloc [128,NT] bf16)."""
    import heapq
    n_blocks = (core_rows + 127) // 128
    NBLK = n_cores * n_blocks
    last_blk_rows = core_rows - (n_blocks - 1) * 128
    cap0 = np.full(NBLK, 128, np.int64)
    cap0[n_blocks - 1::n_blocks] = last_blk_rows

    deg = np.bincount(dst, minlength=n_nodes).astype(np.int64)
    order = np.argsort(-deg, kind='stable')
    nonself = src != dst
    src = src[nonself]
    dst = dst[nonself]

    assign_blk = np.empty(n_nodes, np.int64)
    slot_of = np.empty(n_nodes, np.int64)
    cap = cap0.copy()
    fill = np.zeros(NBLK, np.int64)
    heap = [(0, b) for b in range(NBLK)]
    heapq.heapify(heap)
    for n in order:
        s, b = heapq.heappop(heap)
        assign_blk[n] = b
        slot_of[n] = fill[b]
        fill[b] += 1
        cap[b] -= 1
        if cap[b]:
            heapq.heappush(heap, (s + deg[n], b))

    core_of_blk = assign_blk // n_blocks
    lblk_of = assign_blk % n_blocks
    perm_pos = core_of_blk * core_rows + lblk_of * 128 + slot_of

    eblk = assign_blk[dst]
    cnt = np.bincount(eblk, minlength=NBLK)
    T_blk = int(np.ceil(cnt.max() / 128.0))
    capE = T_blk * 128

    # slot edges into [NBLK, T_blk*128] padded arrays, sorted by src within
    # each block so early tiles only need early table rows
    order_e = np.lexsort((src, eblk))
    se, de = src[order_e], dst[order_e]
    blk_sorted = eblk[order_e]
    starts = np.zeros(NBLK + 1, np.int64)
    np.cumsum(cnt, out=starts[1:])
    sidx = np.zeros((NBLK, capE), np.int32)
    didx = np.zeros((NBLK, capE), np.int32)
    dloc = np.full((NBLK, capE), -1.0, np.float32)
    pos_in_blk = np.arange(len(se)) - starts[blk_sorted]
    sidx[blk_sorted, pos_in_blk] = se
    didx[blk_sorted, pos_in_blk] = de
    dloc[blk_sorted, pos_in_blk] = slot_of[de]

    # per-(block, tile) max src row, chunk-quantized, maxed across cores (SPMD)
    CHROWS = 2048
    s3 = sidx.reshape(NBLK, T_blk, 128)
    tmax = s3.max(axis=2)                                   # [NBLK, T_blk]
    tmax = np.maximum.accumulate(tmax, axis=1)
    tmax = tmax.reshape(n_cores, n_blocks, T_blk).max(axis=0)   # [n_blocks, T_blk]
    tile_rows = np.minimum((tmax // CHROWS + 1) * CHROWS, n_nodes).astype(np.int64)

    out = []
    for c in range(n_cores):
        lo, hi = c * n_blocks, (c + 1) * n_blocks
        # [n_blocks, T_blk, 128] -> [128, n_blocks*T_blk]
        s_c = sidx[lo:hi].reshape(n_blocks * T_blk, 128).T
        l2 = dloc[lo:hi].reshape(n_blocks * T_blk, 128)
        out.append({"srcidx": np.ascontiguousarray(s_c),
                    "dstloc": np.ascontiguousarray(l2.T.astype(bf16)),
                    "dstlocT": np.ascontiguousarray(l2.astype(bf16))})
    return T_blk, n_blocks, perm_pos, tile_rows, out


def scale_routed(routed, tcols, fout, hh, n_cores, core_rows, n_blocks):
    """Pre-multiply gather offsets by the table row pitch (flat-table gathers);
    build per-core adrow offsets pointing at each block's al_dst columns."""
    last = core_rows - (n_blocks - 1) * 128
    out = []
    for c, r in enumerate(routed):
        adrow = np.zeros((128, n_blocks), np.int32)
        for b in range(n_blocks):
            brows = 128 if b < n_blocks - 1 else last
            pp = np.minimum(np.arange(128), brows - 1)
            adrow[:, b] = (c * core_rows + b * 128 + pp) * tcols
        out.append({"srcidx": r["srcidx"] * np.int32(tcols),
                    "dstloc": r["dstloc"], "dstlocT": r["dstlocT"],
                    "adrow": adrow})
    return out


MAX_WAITS = 1


def fix_excess_waits(nc):
    """Post-process BIR JSON: any instruction with >MAX_WAITS sem-waits gets
    preceding Nop instructions carrying the excess waits (same engine, in-order).
    Monkeypatches nc.to_json_bytes to return the fixed JSON."""
    raw = nc.to_json_bytes()
    d = json.loads(raw)
    n_fix = 0
    for f in d["functions"]:
        for bb in f["blocks"]:
            out = []
            for inst in bb["instructions"]:
                si = inst.get("sync_info")
                waits = (si or {}).get("on_wait") or []
                if len(waits) > MAX_WAITS:
                    extra = waits[:-MAX_WAITS]
                    keep = waits[-MAX_WAITS:]
                    for ci in range(0, len(extra), MAX_WAITS):
                        chunk = extra[ci:ci + MAX_WAITS]
                        n_fix += 1
                        out.append({
                            "debug": inst.get("debug", 0),
                            "engine": inst["engine"],
                            "ins": [],
                            "is_reset_sema": False,
                            "name": f"{inst['name']}-wfix{ci}",
                            "opcode": "EventSemaphore",
                            "outs": [],
                            "sync_info": {"on_update": [], "on_wait": chunk},
                        })
                    si["on_wait"] = keep
                out.append(inst)
            bb["instructions"] = out
    fixed = json.dumps(d).encode()
    nc.to_json_bytes = lambda: fixed
    return n_fix


# ---------------- top-level kernel ----------------

N_NODES = 50000
N_CORES = 8
CORE_ROWS = N_NODES // N_CORES
_CACHE = {}


def _get_program(key, builder):
    if key not in _CACHE:
        nc = builder()
        fix_excess_waits(nc)
        _CACHE[key] = nc
    return _CACHE[key]


def _make_bcat(b, H, C, TCOLS, mode):
    """Per-row bias/128 for the ones-matmul: [b' | 0 | 0] tiled to 128 rows."""
    b = np.asarray(b, np.float32)
    row = np.zeros(TCOLS, np.float32)
    if mode == "elu":
        row[:H * C] = b
    else:
        row[:H * C] = np.tile(b, H)
    return np.tile((row / 128.0)[None, :], (128, 1)).astype(bf16)


def kernel(x, edge_index, W1, a_src1, a_dst1, b1, W2, a_src2, a_dst2, b2):
    from concourse.bass_utils import run_bass_kernel_spmd

    x = np.asarray(x, np.float32)
    ei = np.asarray(edge_index)
    N = N_NODES
    src = np.concatenate([ei[0], np.arange(N)]).astype(np.int64)
    dst = np.concatenate([ei[1], np.arange(N)]).astype(np.int64)
    T_blk, n_blocks, perm_pos, tile_rows, routed = route_edges(
        src, dst, N_CORES, CORE_ROWS, N)
    tr_key = hash(tile_rows.tobytes())

    iota_rows = np.tile(np.arange(128, dtype=np.float32)[None, :], (128, 1)).astype(bf16)
    iota_col = np.arange(128, dtype=np.float32)[:, None].astype(bf16)
    ones_sq = np.ones((128, 128), np.float32).astype(bf16)

    # ---- layer 1 ----
    H1, C1 = 8, 32
    Wcat1 = fold_weights(W1, a_src1, a_dst1, H1, C1)
    wb1 = bool(np.any(np.asarray(b1, np.float32) != 0))
    nc1 = _get_program(("l1", T_blk, wb1, tr_key), lambda: build_gat_layer(
        N, 128, H1, C1, T_blk, n_blocks, CORE_ROWS, "elu", with_bias=wb1,
        tile_rows=tile_rows))
    com1 = {
        "xT": pack_kdim(np.ascontiguousarray(x.T)).astype(bf16),
        "wcat": pack_kdim(Wcat1).astype(bf16),
        "bcat": _make_bcat(b1, H1, C1, Wcat1.shape[1], "elu"),
        "iota": iota_rows, "iotac": iota_col, "ones": ones_sq,
    }
    routed1 = scale_routed(routed, Wcat1.shape[1], H1 * C1, H1,
                           N_CORES, CORE_ROWS, n_blocks)
    in_maps1 = [dict(com1, **routed1[c]) for c in range(N_CORES)]
    res1 = run_bass_kernel_spmd(nc1, in_maps1, list(range(N_CORES)))
    h1p = np.concatenate([np.asarray(res1.results[c]["out"]) for c in range(N_CORES)],
                         axis=0)
    h1 = np.asarray(h1p, np.float32)[perm_pos]          # unpermute to node order

    # ---- layer 2 ----
    H2, C2 = 8, 40
    Wcat2 = fold_weights(W2, a_src2, a_dst2, H2, C2)
    wb2 = bool(np.any(np.asarray(b2, np.float32) != 0))
    nc2 = _get_program(("l2", T_blk, wb2, tr_key), lambda: build_gat_layer(
        N, 256, H2, C2, T_blk, n_blocks, CORE_ROWS, "mean_lsm", with_bias=wb2,
        tile_rows=tile_rows))
    com2 = {
        "xT": pack_kdim(np.ascontiguousarray(h1.T)).astype(bf16),
        "wcat": pack_kdim(Wcat2).astype(bf16),
        "bcat": _make_bcat(b2, H2, C2, Wcat2.shape[1], "mean_lsm"),
        "iota": iota_rows, "iotac": iota_col, "ones": ones_sq,
    }
    routed2 = scale_routed(routed, Wcat2.shape[1], H2 * C2, H2,
                           N_CORES, CORE_ROWS, n_blocks)
    in_maps2 = [dict(com2, **routed2[c]) for c in range(N_CORES)]
    res2 = run_bass_kernel_spmd(nc2, in_maps2, list(range(N_CORES)))
    outp = np.concatenate([np.asarray(res2.results[c]["out"]) for c in range(N_CORES)],
                          axis=0)
    return np.asarray(outp, np.float32)[perm_pos]



# revision 33
# speedup vs baseline: 2.3245x; 2.3245x over previous
"""GAT (2-layer, PyG-style) forward on 8 TRN2 NeuronCores.

Sharding: dst-node blocks across cores (host-permuted for per-block edge-count
balance); per-core edge lists routed by dst block on host; self-loops folded
into the edge list. Per dst block of 128 nodes:
  - one batched indirect DMA gathers the h-rows of all T_blk*128 edge sources
    (table rows are c-major per head so the alpha*h product runs in DVE 2x),
  - two tiny indirect DMAs fetch al_dst[dst] and accumulate al_src[src]
    (compute_op=add) per edge,
  - the 0/1 dst-slot selection matrix s01 is produced by gathering rows of a
    small identity table and/or is_equal compares (tunable DVE/Pool/DMA split),
  - segment softmax numerator+denominator come from one PE matmul per edge
    tile accumulating [alpha*h | alpha] into PSUM,
  - epilogues (div by denom + ELU / head-mean + log-softmax) run batched over
    groups of 4 blocks to amortize per-instruction engine overheads.
Dense phases (x @ Wcat -> node table in DRAM) are replicated per core; PSUM
-> SBUF bf16 conversion copies round-robin across Act/DVE/Pool."""
import sys
if '/opt/trn_rl_repo' not in sys.path:
    sys.path.insert(0, '/opt/trn_rl_repo')
import json
import numpy as np
import ml_dtypes

import concourse.bass as bass
import concourse.mybir as mybir
import concourse.tile as tile

bf16 = ml_dtypes.bfloat16
F32 = mybir.dt.float32
BF16 = mybir.dt.bfloat16
I32 = mybir.dt.int32
ALU = mybir.AluOpType
ACTF = mybir.ActivationFunctionType

IDROWS = 130                    # identity table rows (128 one-hot + 2 zero)


def mkap(ap, dims, elem_offset=0):
    """AP with explicit [step, count] free dims (elements) after the partition dim."""
    return bass.AP(ap.tensor, ap.offset + elem_offset,
                   [list(ap.ap[0])] + [list(d) for d in dims])


def build_gat_layer(N, FIN, H, C, T_blk, n_blocks, core_rows, mode,
                    with_bias=False, s01_split=(0, 0), G=4, ST=8,
                    pre_s01=0, pre_split=None, copy_rr=("act", "dve"),
                    u_eng="dve", sm_eng="dve", lr_eng="dve",
                    g_bufs=3, x_bufs=3, mp_bufs=3, mp_pool=0):
    """mode: 'elu' (layer 1) or 'mean_lsm' (layer 2).

    s01_split = (n_dve, n_pool): per block, the first T-n_dve-n_pool tiles of
    the selection matrix come from the identity-gather, then n_dve tiles via
    is_equal on DVE, then n_pool on Pool. The first pre_s01 blocks' s01 are
    built early (interleaved into the dense phase) with pre_split."""
    FOUT = H * C
    TCOLS = FOUT + 2 * H          # table row: [h(c-major) | al_src | al_dst]
    GC = FOUT + H                 # matmul rhs per tile: [mp | ee]
    NT = n_blocks * T_blk
    KCH = FIN // 128
    n_dve, n_pool = s01_split
    n_gath = T_blk - n_dve - n_pool
    assert n_gath >= 0
    if pre_split is None:
        pre_split = (T_blk, 0)
    p_dve, p_pool = pre_split
    p_gath = T_blk - p_dve - p_pool
    assert p_gath >= 0
    # neuronx-cc ISA: Pool supports only add/mult/copy — no is_equal, no
    # scalar_tensor_tensor, and no PSUM access
    assert n_pool == 0 and p_pool == 0
    assert u_eng == "dve" and sm_eng == "dve" and lr_eng == "dve"
    assert all(e in ("act", "dve") for e in copy_rr)
    any_gath = (n_gath and pre_s01 < n_blocks) or (p_gath and pre_s01)
    any_cmp = n_dve or n_pool or (pre_s01 and (p_dve or p_pool))

    nc = bass.Bass("TRN2", target_bir_lowering=False, debug=False, num_devices=8)

    xT = nc.dram_tensor("xT", [128, KCH, N], BF16, kind="ExternalInput")
    wcat = nc.dram_tensor("wcat", [128, KCH, TCOLS], BF16, kind="ExternalInput")
    srcoff_in = nc.dram_tensor("srcoff", [128, NT], I32, kind="ExternalInput")
    aldoff_in = nc.dram_tensor("aldoff", [128, NT], I32, kind="ExternalInput")
    alsoff_in = nc.dram_tensor("alsoff", [128, NT], I32, kind="ExternalInput")
    if any_gath:
        s01off_in = nc.dram_tensor("s01off", [128, NT], I32, kind="ExternalInput")
        ident_in = nc.dram_tensor("ident", [IDROWS, 128], BF16, kind="ExternalInput")
    if any_cmp:
        dstloc_in = nc.dram_tensor("dstloc", [128, NT], BF16, kind="ExternalInput")
        iota_in = nc.dram_tensor("iota", [128, 128], BF16, kind="ExternalInput")
    if with_bias:
        bcat_in = nc.dram_tensor("bcat", [128, TCOLS], BF16, kind="ExternalInput")
        ones_in = nc.dram_tensor("ones", [128, 128], BF16, kind="ExternalInput")
    if mode == "elu":
        OCOLS = FOUT
        ODT = BF16
    else:
        OCOLS = C
        ODT = F32
    out_d = nc.dram_tensor("out", [core_rows, OCOLS], ODT, kind="ExternalOutput")
    table = nc.dram_tensor("table", [N, TCOLS], BF16)

    CH = ST * 128                 # xT chunk columns
    n_groups = (n_blocks + G - 1) // G

    with tile.TileContext(nc) as tc:
        with (
            tc.tile_pool(name="const", bufs=1) as kpool,
            tc.tile_pool(name="xchunk", bufs=x_bufs) as xpool,
            tc.tile_pool(name="stage", bufs=3) as stpool,
            tc.tile_pool(name="g", bufs=g_bufs) as gpool,
            tc.tile_pool(name="sf", bufs=2) as sfpool,
            tc.tile_pool(name="s01", bufs=max(3, pre_s01 + 3)) as spool,
            tc.tile_pool(name="mp", bufs=mp_bufs) as mppool,
            tc.tile_pool(name="epi", bufs=2) as epool,
            tc.tile_pool(name="oacc", bufs=1) as opool,
        ):
            # ---- constants ----
            wcat_sb = kpool.tile([128, KCH * TCOLS], BF16)
            nc.sync.dma_start(out=wcat_sb[:], in_=wcat[:].rearrange("p k c -> p (k c)"))
            srcoff_sb = kpool.tile([128, NT], I32)
            nc.sync.dma_start(out=srcoff_sb[:], in_=srcoff_in[:])
            aldoff_sb = kpool.tile([128, NT], I32)
            nc.sync.dma_start(out=aldoff_sb[:], in_=aldoff_in[:])
            alsoff_sb = kpool.tile([128, NT], I32)
            nc.sync.dma_start(out=alsoff_sb[:], in_=alsoff_in[:])
            if any_gath:
                s01off_sb = kpool.tile([128, NT], I32)
                nc.sync.dma_start(out=s01off_sb[:], in_=s01off_in[:])
            if any_cmp:
                dstloc_sb = kpool.tile([128, NT], BF16)
                nc.sync.dma_start(out=dstloc_sb[:], in_=dstloc_in[:])
                iota_sb = kpool.tile([128, 128], BF16)
                nc.sync.dma_start(out=iota_sb[:], in_=iota_in[:])
            if with_bias:
                bcat_sb = kpool.tile([128, TCOLS], BF16)
                nc.sync.dma_start(out=bcat_sb[:], in_=bcat_in[:])
                ones_sb = kpool.tile([128, 128], BF16)
                nc.sync.dma_start(out=ones_sb[:], in_=ones_in[:])
            neg1 = kpool.tile([128, 1], F32)
            nc.vector.memset(neg1[:], -1.0)

            obuf = opool.tile([128, n_blocks * OCOLS], ODT)

            table_flat = bass.AP(table[:].tensor, 0,
                                 [[N * TCOLS, 1], [1, N * TCOLS]])
            if n_gath:
                ident_flat = bass.AP(ident_in[:].tensor, 0,
                                     [[IDROWS * 128, 1], [1, IDROWS * 128]])

            # ---- s01 build helper ----
            s01_pre = {}

            def build_s01(b, split):
                bT = b * T_blk
                sd, sp = split
                sg = T_blk - sd - sp
                s01 = spool.tile([128, T_blk * 128], BF16, tag="s01")
                t0 = 0
                if sg:
                    nc.gpsimd.indirect_dma_start(
                        out=s01[:, 0:sg * 128], out_offset=None,
                        in_=ident_flat,
                        in_offset=bass.IndirectOffsetOnAxis(
                            ap=s01off_sb[:, bT:bT + sg], axis=1),
                    )
                    t0 = sg
                if sd:
                    nc.vector.tensor_tensor(
                        out=s01[:, t0 * 128:(t0 + sd) * 128],
                        in0=mkap(dstloc_sb[:], [[1, sd], [0, 128]],
                                 elem_offset=bT + t0),
                        in1=mkap(iota_sb[:], [[0, sd], [1, 128]]),
                        op=ALU.is_equal,
                    )
                    t0 += sd
                if sp:
                    nc.gpsimd.tensor_tensor(
                        out=s01[:, t0 * 128:(t0 + sp) * 128],
                        in0=mkap(dstloc_sb[:], [[1, sp], [0, 128]],
                                 elem_offset=bT + t0),
                        in1=mkap(iota_sb[:], [[0, sp], [1, 128]]),
                        op=ALU.is_equal,
                    )
                return s01

            # ---- dense phase: table[N, TCOLS] = x @ Wcat (+ b') ----
            with tc.tile_pool(name="dpsum", bufs=2, space="PSUM") as dppool:
                n_ch = (N + CH - 1) // CH
                pre_every = max(1, n_ch // pre_s01) if pre_s01 else 0
                cp_i = 0
                for ci in range(n_ch):
                    if pre_every and ci % pre_every == 0 and len(s01_pre) < pre_s01:
                        b = len(s01_pre)
                        s01_pre[b] = build_s01(b, pre_split)
                    c0 = ci * CH
                    ccols = min(CH, N - c0)
                    nt_ch = (ccols + 127) // 128
                    xc = xpool.tile([128, KCH * CH], BF16, tag="xc")
                    nc.sync.dma_start(
                        out=mkap(xc[:], [[CH, KCH], [1, ccols]]),
                        in_=xT[:, :, c0:c0 + ccols],
                    )
                    st = stpool.tile([128, ST * TCOLS], BF16, tag="st")
                    for pr in range((nt_ch + 1) // 2):
                        psum = dppool.tile([128, 1024], F32, tag="dp")
                        nsub = min(2, nt_ch - pr * 2)
                        for sub in range(nsub):
                            tl = pr * 2 + sub
                            col = tl * 128
                            rows = min(128, ccols - col)
                            for k in range(KCH):
                                nc.tensor.matmul(
                                    psum[:rows, sub * 512: sub * 512 + TCOLS],
                                    lhsT=xc[:, k * CH + col: k * CH + col + rows],
                                    rhs=wcat_sb[:, k * TCOLS:(k + 1) * TCOLS],
                                    start=(k == 0),
                                    stop=(not with_bias and k == KCH - 1),
                                )
                            if with_bias:
                                nc.tensor.matmul(
                                    psum[:rows, sub * 512: sub * 512 + TCOLS],
                                    lhsT=ones_sb[:, 0:rows],
                                    rhs=bcat_sb[:],
                                    start=False, stop=True,
                                )
                        dst_ap = mkap(st[:], [[TCOLS, nsub], [1, TCOLS]],
                                      elem_offset=pr * 2 * TCOLS)
                        src_ap = mkap(psum[:], [[512, nsub], [1, TCOLS]])
                        # Pool/gpsimd cannot read PSUM: copies go to Act/DVE only
                        eng = copy_rr[cp_i % len(copy_rr)]
                        cp_i += 1
                        if eng == "act":
                            nc.scalar.copy(out=dst_ap, in_=src_ap)
                        else:
                            nc.vector.tensor_copy(out=dst_ap, in_=src_ap)
                    # flush: full 128-row tiles in one strided DMA, tail separately
                    n_full = ccols // 128
                    if n_full:
                        nc.sync.dma_start(
                            out=table[c0:c0 + n_full * 128, :]
                            .rearrange("(a p) c -> p a c", p=128),
                            in_=mkap(st[:], [[TCOLS, n_full], [1, TCOLS]]),
                        )
                    rem = ccols - n_full * 128
                    if rem:
                        nc.sync.dma_start(
                            out=table[c0 + n_full * 128: c0 + ccols, :],
                            in_=st[:rem, n_full * TCOLS:(n_full + 1) * TCOLS],
                        )

            # ---- edge phase: groups of G blocks ----
            with tc.tile_pool(name="upsum", bufs=2, space="PSUM") as uppool:
                def group_front(gi):
                    b0 = gi * G
                    gb = min(G, n_blocks - b0)
                    gT = gb * T_blk
                    o0 = b0 * T_blk
                    # per-edge attention logits: al_dst[dst] then += al_src[src]
                    sf = sfpool.tile([128, G * T_blk * H], BF16, tag="sf")
                    nc.gpsimd.indirect_dma_start(
                        out=sf[:, 0:gT * H], out_offset=None, in_=table_flat,
                        in_offset=bass.IndirectOffsetOnAxis(
                            ap=aldoff_sb[:, o0:o0 + gT], axis=1),
                    )
                    nc.gpsimd.indirect_dma_start(
                        out=sf[:, 0:gT * H], out_offset=None, in_=table_flat,
                        in_offset=bass.IndirectOffsetOnAxis(
                            ap=alsoff_sb[:, o0:o0 + gT], axis=1),
                        compute_op=ALU.add,
                    )
                    lr = sfpool.tile([128, G * T_blk * H], BF16, tag="lr")
                    lr_e = nc.vector if lr_eng == "dve" else nc.gpsimd
                    lr_e.scalar_tensor_tensor(
                        out=lr[:, 0:gT * H], in0=sf[:, 0:gT * H], scalar=0.2,
                        in1=sf[:, 0:gT * H], op0=ALU.mult, op1=ALU.max)
                    up = uppool.tile([128, G * 512], F32, tag="up")
                    gtiles, mtiles, stiles = [], [], []
                    for q in range(gb):
                        b = b0 + q
                        bT = b * T_blk
                        # payload gather: h rows (c-major) of all edge sources
                        g_blk = gpool.tile([128, T_blk * FOUT], BF16, tag="g")
                        nc.gpsimd.indirect_dma_start(
                            out=g_blk[:], out_offset=None, in_=table_flat,
                            in_offset=bass.IndirectOffsetOnAxis(
                                ap=srcoff_sb[:, bT:bT + T_blk], axis=1),
                        )
                        gtiles.append(g_blk)
                        # selection matrix s01[j, (t,d)] = (dstloc[j,t] == d)
                        if b in s01_pre:
                            s01 = s01_pre.pop(b)
                        else:
                            s01 = build_s01(b, s01_split)
                        stiles.append(s01)
                        # mpee[:, t*GC:] = [ee*h (c-major) | ee]
                        mpee = mppool.tile([128, T_blk * GC], BF16, tag="mpee")
                        nc.scalar.activation(
                            mkap(mpee[:], [[GC, T_blk], [1, H]], elem_offset=FOUT),
                            lr[:, q * T_blk * H:(q + 1) * T_blk * H],
                            ACTF.Exp)
                        n_mpv = T_blk - mp_pool
                        nc.vector.tensor_tensor(
                            out=mkap(mpee[:], [[GC, n_mpv], [1, FOUT]]),
                            in0=g_blk[:, 0:n_mpv * FOUT],
                            in1=mkap(mpee[:], [[GC, n_mpv], [0, C], [1, H]],
                                     elem_offset=FOUT),
                            op=ALU.mult,
                        )
                        if mp_pool:
                            nc.gpsimd.tensor_tensor(
                                out=mkap(mpee[:], [[GC, mp_pool], [1, FOUT]],
                                         elem_offset=n_mpv * GC),
                                in0=g_blk[:, n_mpv * FOUT:T_blk * FOUT],
                                in1=mkap(mpee[:], [[GC, mp_pool], [0, C], [1, H]],
                                         elem_offset=n_mpv * GC + FOUT),
                                op=ALU.mult,
                            )
                        mtiles.append(mpee)
                    for q in range(gb):
                        s01 = stiles[q]
                        mpee = mtiles[q]
                        for t in range(T_blk):
                            nc.tensor.matmul(
                                up[:, q * 512: q * 512 + GC],
                                lhsT=s01[:, t * 128:(t + 1) * 128],
                                rhs=mpee[:, t * GC:(t + 1) * GC],
                                start=(t == 0),
                                stop=(t == T_blk - 1),
                            )
                    return up

                def group_epi(gi, up):
                    b0 = gi * G
                    gb = min(G, n_blocks - b0)
                    rec = epool.tile([128, G * H], F32, tag="rec")
                    nc.vector.reciprocal(
                        out=rec[:, 0:gb * H],
                        in_=mkap(up[:], [[512, gb], [1, H]], elem_offset=FOUT))
                    # NOTE: up lives in PSUM; only DVE/Act may touch PSUM
                    u = epool.tile([128, G * FOUT], F32, tag="u")
                    nc.vector.tensor_tensor(
                        out=u[:, 0:gb * FOUT],
                        in0=mkap(up[:], [[512, gb], [1, FOUT]]),
                        in1=mkap(rec[:], [[H, gb], [0, C], [1, H]]),
                        op=ALU.mult)
                    if mode == "elu":
                        nr = epool.tile([128, G * FOUT], F32, tag="nr")
                        nc.scalar.activation(nr[:, 0:gb * FOUT], u[:, 0:gb * FOUT],
                                             ACTF.Relu, scale=-1.0)
                        ex = epool.tile([128, G * FOUT], F32, tag="ex")
                        nc.scalar.activation(ex[:, 0:gb * FOUT], nr[:, 0:gb * FOUT],
                                             ACTF.Exp, scale=-1.0)
                        sm = epool.tile([128, G * FOUT], F32, tag="sm")
                        sm_e = nc.vector if sm_eng == "dve" else nc.gpsimd
                        sm_e.scalar_tensor_tensor(
                            out=sm[:, 0:gb * FOUT], in0=u[:, 0:gb * FOUT],
                            scalar=0.0, in1=ex[:, 0:gb * FOUT],
                            op0=ALU.max, op1=ALU.add)
                        nc.scalar.activation(
                            obuf[:, b0 * OCOLS:(b0 + gb) * OCOLS],
                            sm[:, 0:gb * FOUT],
                            ACTF.Identity, bias=neg1[:])
                    else:
                        # mean over heads (c-major: pairwise over inner h)
                        m4 = epool.tile([128, G * C * 4], F32, tag="m4")
                        nc.vector.tensor_tensor(
                            out=m4[:, 0:gb * C * 4],
                            in0=mkap(u[:], [[FOUT, gb], [H, C], [1, 4]]),
                            in1=mkap(u[:], [[FOUT, gb], [H, C], [1, 4]],
                                     elem_offset=4),
                            op=ALU.add)
                        m2 = epool.tile([128, G * C * 2], F32, tag="m2")
                        nc.vector.tensor_tensor(
                            out=m2[:, 0:gb * C * 2],
                            in0=mkap(m4[:], [[C * 4, gb], [4, C], [1, 2]]),
                            in1=mkap(m4[:], [[C * 4, gb], [4, C], [1, 2]],
                                     elem_offset=2),
                            op=ALU.add)
                        m1 = epool.tile([128, G * C], F32, tag="m1")
                        nc.vector.tensor_tensor(
                            out=m1[:, 0:gb * C],
                            in0=mkap(m2[:], [[C * 2, gb], [2, C]]),
                            in1=mkap(m2[:], [[C * 2, gb], [2, C]], elem_offset=1),
                            op=ALU.add)
                        mx = epool.tile([128, G], F32, tag="mx")
                        nc.vector.reduce_max(
                            out=mx[:, 0:gb],
                            in_=mkap(m1[:], [[C, gb], [1, C]]),
                            axis=mybir.AxisListType.X)
                        nmx = epool.tile([128, G], F32, tag="nmx")
                        nc.vector.tensor_scalar_mul(out=nmx[:, 0:gb],
                                                    in0=mx[:, 0:gb],
                                                    scalar1=-1.0 / H)
                        exs = epool.tile([128, C], F32, tag="exs")
                        sms = epool.tile([128, G], F32, tag="sms")
                        for q in range(gb):
                            nc.scalar.activation(
                                exs[:], m1[:, q * C:(q + 1) * C], ACTF.Exp,
                                scale=1.0 / H, bias=nmx[:, q:q + 1],
                                accum_out=sms[:, q:q + 1])
                        lg = epool.tile([128, G], F32, tag="lg")
                        nc.scalar.activation(lg[:, 0:gb], sms[:, 0:gb], ACTF.Ln)
                        nb = epool.tile([128, G], F32, tag="nb")
                        nc.vector.tensor_tensor(out=nb[:, 0:gb], in0=nmx[:, 0:gb],
                                                in1=lg[:, 0:gb], op=ALU.subtract)
                        for q in range(gb):
                            nc.scalar.activation(
                                obuf[:, (b0 + q) * OCOLS:(b0 + q + 1) * OCOLS],
                                m1[:, q * C:(q + 1) * C],
                                ACTF.Identity, scale=1.0 / H, bias=nb[:, q:q + 1])

                    # flush this group's output rows (tail block separately);
                    # runs for both modes
                    nfull_g = gb - 1 if b0 + gb == n_blocks else gb
                    if nfull_g:
                        nc.sync.dma_start(
                            out=out_d[b0 * 128:(b0 + nfull_g) * 128, :]
                            .rearrange("(a p) c -> p a c", p=128),
                            in_=mkap(obuf[:], [[OCOLS, nfull_g], [1, OCOLS]],
                                     elem_offset=b0 * OCOLS),
                        )
                    if b0 + gb == n_blocks:
                        lbr = core_rows - (n_blocks - 1) * 128
                        nc.sync.dma_start(
                            out=out_d[(n_blocks - 1) * 128:core_rows, :],
                            in_=obuf[:lbr, (n_blocks - 1) * OCOLS:
                                     n_blocks * OCOLS],
                        )

                prev = None
                for gi in range(n_groups):
                    up = group_front(gi)
                    if prev is not None:
                        group_epi(gi - 1, prev)
                    prev = up
                group_epi(n_groups - 1, prev)

    return nc


# ---------------- host side ----------------

def fold_weights_cmajor(W, a_src, a_dst, H, C, in_perm=None):
    """Wcat [FIN, C*H + 2H] f32: [W.T cols c-major | Wa_src | Wa_dst].

    in_perm permutes the FIN axis (to match a c-major input layout)."""
    WT = np.asarray(W, np.float32).T.copy()           # [FIN, H*C]
    FIN = WT.shape[0]
    W3 = WT.reshape(FIN, H, C)
    Wc = np.ascontiguousarray(W3.transpose(0, 2, 1)).reshape(FIN, C * H)
    Wa_s = np.einsum('fhc,hc->fh', W3, np.asarray(a_src, np.float32))
    Wa_d = np.einsum('fhc,hc->fh', W3, np.asarray(a_dst, np.float32))
    M = np.concatenate([Wc, Wa_s, Wa_d], axis=1)
    if in_perm is not None:
        M = M[in_perm]
    return M


def pack_kdim(M):
    """[FIN, COLS] -> [128, KCH, COLS]: row k*128+p -> [p, k]."""
    FIN, COLS = M.shape
    KCH = FIN // 128
    return np.ascontiguousarray(M.reshape(KCH, 128, COLS).transpose(1, 0, 2))


def route_edges(src, dst, n_cores, core_rows, n_nodes):
    """Balanced dst-node placement + per-core edge routing (self-loops kept).

    Returns (T_blk, n_blocks, perm_pos[n_nodes], per-core dict of index
    arrays [128, NT])."""
    import heapq
    n_blocks = (core_rows + 127) // 128
    NBLK = n_cores * n_blocks
    last_blk_rows = core_rows - (n_blocks - 1) * 128
    cap0 = np.full(NBLK, 128, np.int64)
    cap0[n_blocks - 1::n_blocks] = last_blk_rows

    deg = np.bincount(dst, minlength=n_nodes).astype(np.int64)
    order = np.argsort(-deg, kind='stable')

    assign_blk = np.empty(n_nodes, np.int64)
    slot_of = np.empty(n_nodes, np.int64)
    cap = cap0.copy()
    fill = np.zeros(NBLK, np.int64)
    heap = [(0, b) for b in range(NBLK)]
    heapq.heapify(heap)
    for n in order:
        s, b = heapq.heappop(heap)
        assign_blk[n] = b
        slot_of[n] = fill[b]
        fill[b] += 1
        cap[b] -= 1
        if cap[b]:
            heapq.heappush(heap, (s + deg[n], b))

    core_of_blk = assign_blk // n_blocks
    lblk_of = assign_blk % n_blocks
    perm_pos = core_of_blk * core_rows + lblk_of * 128 + slot_of

    eblk = assign_blk[dst]
    cnt = np.bincount(eblk, minlength=NBLK)
    # fake edges to fill the unused slots of each core's last block, so no
    # dst slot has an empty softmax denominator (avoids inf/NaN lanes)
    n_fake = 128 - last_blk_rows
    need = cnt.copy()
    if n_fake:
        need[n_blocks - 1::n_blocks] += n_fake
    T_blk = int(np.ceil(need.max() / 128.0))
    capE = T_blk * 128

    order_e = np.lexsort((src, eblk))
    se, de = src[order_e], dst[order_e]
    blk_sorted = eblk[order_e]
    starts = np.zeros(NBLK + 1, np.int64)
    np.cumsum(cnt, out=starts[1:])
    sidx = np.zeros((NBLK, capE), np.int32)
    didx = np.zeros((NBLK, capE), np.int32)
    dloc = np.full((NBLK, capE), -1, np.int32)
    pos_in_blk = np.arange(len(se)) - starts[blk_sorted]
    sidx[blk_sorted, pos_in_blk] = se
    didx[blk_sorted, pos_in_blk] = de
    dloc[blk_sorted, pos_in_blk] = slot_of[de]
    if n_fake:
        lastb = np.arange(n_blocks - 1, NBLK, n_blocks)
        for b in lastb:
            e0 = cnt[b]
            dloc[b, e0:e0 + n_fake] = np.arange(last_blk_rows, 128)
            # didx stays 0: al_dst read from row 0, harmless

    out = []
    for c in range(n_cores):
        lo, hi = c * n_blocks, (c + 1) * n_blocks
        # [n_blocks, T_blk, 128] -> [128, n_blocks*T_blk]
        def core_arr(a):
            return np.ascontiguousarray(a[lo:hi].reshape(n_blocks * T_blk, 128).T)
        out.append({"sidx": core_arr(sidx), "didx": core_arr(didx),
                    "dloc": core_arr(dloc)})
    return T_blk, n_blocks, perm_pos, out


def index_inputs(routed_core, tcols, fout, hh):
    """Per-core gather-offset arrays from routed sidx/didx/dloc."""
    sidx = routed_core["sidx"].astype(np.int64)
    didx = routed_core["didx"].astype(np.int64)
    dloc = routed_core["dloc"].astype(np.int64)
    srcoff = (sidx * tcols).astype(np.int32)
    aldoff = (didx * tcols + fout + hh).astype(np.int32)
    alsoff = (sidx * tcols + fout).astype(np.int32)
    s01off = np.where(dloc < 0, 128 * 128, dloc * 128).astype(np.int32)
    dstloc = dloc.astype(np.float32).astype(bf16)
    return {"srcoff": srcoff, "aldoff": aldoff, "alsoff": alsoff,
            "s01off": s01off, "dstloc": dstloc}


MAX_WAITS = 1


def fix_excess_waits(nc):
    """Post-process BIR JSON: any instruction with >MAX_WAITS sem-waits gets
    preceding Nop instructions carrying the excess waits (same engine, in-order).
    Monkeypatches nc.to_json_bytes to return the fixed JSON."""
    raw = nc.to_json_bytes()
    d = json.loads(raw)
    n_fix = 0
    for f in d["functions"]:
        for bb in f["blocks"]:
            out = []
            for inst in bb["instructions"]:
                si = inst.get("sync_info")
                waits = (si or {}).get("on_wait") or []
                if len(waits) > MAX_WAITS:
                    extra = waits[:-MAX_WAITS]
                    keep = waits[-MAX_WAITS:]
                    for ci in range(0, len(extra), MAX_WAITS):
                        chunk = extra[ci:ci + MAX_WAITS]
                        n_fix += 1
                        out.append({
                            "debug": inst.get("debug", 0),
                            "engine": inst["engine"],
                            "ins": [],
                            "is_reset_sema": False,
                            "name": f"{inst['name']}-wfix{ci}",
                            "opcode": "EventSemaphore",
                            "outs": [],
                            "sync_info": {"on_update": [], "on_wait": chunk},
                        })
                    si["on_wait"] = keep
                out.append(inst)
            bb["instructions"] = out
    fixed = json.dumps(d).encode()
    nc.to_json_bytes = lambda: fixed
    return n_fix


# ---------------- top-level kernel ----------------

N_NODES = 50000
N_CORES = 8
CORE_ROWS = N_NODES // N_CORES
H1, C1 = 8, 32
H2, C2 = 8, 40
# layer tuning: s01 split (n_dve, n_pool) with the rest from identity-gather,
# prebuild window, engine assignments
TUNE1 = dict(s01_split=(9, 0), pre_s01=6, copy_rr=("act",))
TUNE2 = dict(s01_split=(9, 0), pre_s01=6, copy_rr=("act",))
_CACHE = {}


def _get_program(key, builder):
    if key not in _CACHE:
        nc = builder()
        fix_excess_waits(nc)
        _CACHE[key] = nc
    return _CACHE[key]


def _make_bcat(b, H, C, TCOLS, mode):
    """Per-row bias/128 for the ones-matmul (c-major): [b' | 0 | 0]."""
    b = np.asarray(b, np.float32)
    row = np.zeros(TCOLS, np.float32)
    if mode == "elu":
        row[:H * C] = b.reshape(H, C).T.ravel()
    else:
        row[:H * C] = np.tile(b, H).reshape(H, C).T.ravel()
    return np.tile((row / 128.0)[None, :], (128, 1)).astype(bf16)


def kernel(x, edge_index, W1, a_src1, a_dst1, b1, W2, a_src2, a_dst2, b2):
    from concourse.bass_utils import run_bass_kernel_spmd

    x = np.asarray(x, np.float32)
    ei = np.asarray(edge_index)
    N = N_NODES
    src = np.concatenate([ei[0], np.arange(N)]).astype(np.int64)
    dst = np.concatenate([ei[1], np.arange(N)]).astype(np.int64)
    T_blk, n_blocks, perm_pos, routed = route_edges(
        src, dst, N_CORES, CORE_ROWS, N)

    iota_rows = np.tile(np.arange(128, dtype=np.float32)[None, :],
                        (128, 1)).astype(bf16)
    ident = np.zeros((IDROWS, 128), np.float32)
    ident[:128, :128] = np.eye(128)
    ident = ident.astype(bf16)
    ones_sq = np.ones((128, 128), np.float32).astype(bf16)

    # ---- layer 1 ----
    FOUT1 = H1 * C1
    Wcat1 = fold_weights_cmajor(W1, a_src1, a_dst1, H1, C1)
    TC1 = Wcat1.shape[1]
    wb1 = bool(np.any(np.asarray(b1, np.float32) != 0))
    nc1 = _get_program(("l1", T_blk, n_blocks, wb1, str(TUNE1)), lambda: build_gat_layer(
        N, 128, H1, C1, T_blk, n_blocks, CORE_ROWS, "elu", with_bias=wb1,
        **TUNE1))
    com1 = {
        "xT": pack_kdim(np.ascontiguousarray(x.T)).astype(bf16),
        "wcat": pack_kdim(Wcat1).astype(bf16),
        "iota": iota_rows, "ident": ident,
    }
    if wb1:
        com1["bcat"] = _make_bcat(b1, H1, C1, TC1, "elu")
        com1["ones"] = ones_sq
    in_maps1 = [dict(com1, **index_inputs(routed[c], TC1, FOUT1, H1))
                for c in range(N_CORES)]
    in_maps1 = [_filter_inputs(nc1, m) for m in in_maps1]
    res1 = run_bass_kernel_spmd(nc1, in_maps1, list(range(N_CORES)))
    h1p = np.concatenate([np.asarray(res1.results[c]["out"])
                          for c in range(N_CORES)], axis=0)
    h1 = np.asarray(h1p, np.float32)[perm_pos]      # node order, cols (c1,h1)

    # ---- layer 2 ----
    FOUT2 = H2 * C2
    FIN2 = H1 * C1
    # h1 columns are c-major: our column j=c1*H1+h1 is original feature
    # f=h1*C1+c1, so permute Wcat2's input-feature rows to match
    row_perm = (np.arange(FIN2).reshape(H1, C1).T).ravel()
    Wcat2 = fold_weights_cmajor(W2, a_src2, a_dst2, H2, C2)[row_perm]
    TC2 = Wcat2.shape[1]
    wb2 = bool(np.any(np.asarray(b2, np.float32) != 0))
    nc2 = _get_program(("l2", T_blk, n_blocks, wb2, str(TUNE2)), lambda: build_gat_layer(
        N, 256, H2, C2, T_blk, n_blocks, CORE_ROWS, "mean_lsm", with_bias=wb2,
        **TUNE2))
    com2 = {
        "xT": pack_kdim(np.ascontiguousarray(h1.T)).astype(bf16),
        "wcat": pack_kdim(Wcat2).astype(bf16),
        "iota": iota_rows, "ident": ident,
    }
    if wb2:
        com2["bcat"] = _make_bcat(b2, H2, C2, TC2, "mean_lsm")
        com2["ones"] = ones_sq
    in_maps2 = [dict(com2, **index_inputs(routed[c], TC2, FOUT2, H2))
                for c in range(N_CORES)]
    in_maps2 = [_filter_inputs(nc2, m) for m in in_maps2]
    res2 = run_bass_kernel_spmd(nc2, in_maps2, list(range(N_CORES)))
    outp = np.concatenate([np.asarray(res2.results[c]["out"])
                           for c in range(N_CORES)], axis=0)
    return np.asarray(outp, np.float32)[perm_pos]


def _filter_inputs(nc, m):
    import concourse.mybir as mb
    names = {a.memorylocations[0].name for a in nc.m.functions[0].allocations
             if isinstance(a, mb.MemoryLocationSet) and a.kind == "ExternalInput"}
    return {k: v for k, v in m.items() if k in names}


# revision 38
# speedup vs baseline: 2.4365x; 1.0482x over previous
"""GAT (2-layer, PyG-style) forward on 8 TRN2 NeuronCores.

Sharding: dst-node blocks across cores (host-permuted for per-block edge-count
balance); per-core edge lists routed by dst block on host; self-loops folded
into the edge list. Per dst block of 128 nodes:
  - one batched indirect DMA gathers the h-rows of all T_blk*128 edge sources
    (table rows are c-major per head so the alpha*h product runs in DVE 2x),
  - two tiny indirect DMAs fetch al_dst[dst] and accumulate al_src[src]
    (compute_op=add) per edge,
  - the 0/1 dst-slot selection matrix s01 is produced by gathering rows of a
    small identity table and/or is_equal compares (tunable DVE/Pool/DMA split),
  - segment softmax numerator+denominator come from one PE matmul per edge
    tile accumulating [alpha*h | alpha] into PSUM,
  - epilogues (div by denom + ELU / head-mean + log-softmax) run batched over
    groups of 4 blocks to amortize per-instruction engine overheads.
Dense phases (x @ Wcat -> node table in DRAM) are replicated per core; PSUM
-> SBUF bf16 conversion copies round-robin across Act/DVE/Pool."""
import sys
if '/opt/trn_rl_repo' not in sys.path:
    sys.path.insert(0, '/opt/trn_rl_repo')
import json
import numpy as np
import ml_dtypes

import concourse.bass as bass
import concourse.mybir as mybir
import concourse.tile as tile

bf16 = ml_dtypes.bfloat16
F32 = mybir.dt.float32
BF16 = mybir.dt.bfloat16
I32 = mybir.dt.int32
ALU = mybir.AluOpType
ACTF = mybir.ActivationFunctionType

IDROWS = 130                    # identity table rows (128 one-hot + 2 zero)


def mkap(ap, dims, elem_offset=0):
    """AP with explicit [step, count] free dims (elements) after the partition dim."""
    return bass.AP(ap.tensor, ap.offset + elem_offset,
                   [list(ap.ap[0])] + [list(d) for d in dims])


def build_gat_layer(N, FIN, H, C, T_blk, n_blocks, core_rows, mode,
                    with_bias=False, s01_split=(0, 0), G=4, ST=8,
                    pre_s01=0, pre_split=None, copy_rr=("act", "dve"),
                    u_eng="dve", sm_eng="dve", lr_eng="dve",
                    g_bufs=3, x_bufs=3, mp_bufs=3, mp_pool=0, x_fp8=False):
    """mode: 'elu' (layer 1) or 'mean_lsm' (layer 2).

    s01_split = (n_dve, n_pool): per block, the first T-n_dve-n_pool tiles of
    the selection matrix come from the identity-gather, then n_dve tiles via
    is_equal on DVE, then n_pool on Pool. The first pre_s01 blocks' s01 are
    built early (interleaved into the dense phase) with pre_split."""
    FOUT = H * C
    TCOLS = FOUT + 2 * H          # table row: [h(c-major) | al_src | al_dst]
    GC = FOUT + H                 # matmul rhs per tile: [mp | ee]
    NT = n_blocks * T_blk
    KCH = FIN // 128
    n_dve, n_pool = s01_split
    n_gath = T_blk - n_dve - n_pool
    assert n_gath >= 0
    if pre_split is None:
        pre_split = (T_blk, 0)
    p_dve, p_pool = pre_split
    p_gath = T_blk - p_dve - p_pool
    assert p_gath >= 0
    # neuronx-cc ISA: Pool supports only add/mult/copy — no is_equal, no
    # scalar_tensor_tensor, and no PSUM access
    assert n_pool == 0 and p_pool == 0
    assert u_eng == "dve" and sm_eng == "dve" and lr_eng == "dve"
    assert all(e in ("act", "dve") for e in copy_rr)
    any_gath = (n_gath and pre_s01 < n_blocks) or (p_gath and pre_s01)
    any_cmp = n_dve or n_pool or (pre_s01 and (p_dve or p_pool))

    nc = bass.Bass("TRN2", target_bir_lowering=False, debug=False, num_devices=8)

    XDT = mybir.dt.float8e4 if x_fp8 else BF16
    xT = nc.dram_tensor("xT", [128, KCH, N], XDT, kind="ExternalInput")
    wcat = nc.dram_tensor("wcat", [128, KCH, TCOLS], XDT, kind="ExternalInput")
    srcoff_in = nc.dram_tensor("srcoff", [128, NT], I32, kind="ExternalInput")
    aldoff_in = nc.dram_tensor("aldoff", [128, NT], I32, kind="ExternalInput")
    alsoff_in = nc.dram_tensor("alsoff", [128, NT], I32, kind="ExternalInput")
    if any_gath:
        s01off_in = nc.dram_tensor("s01off", [128, NT], I32, kind="ExternalInput")
        ident_in = nc.dram_tensor("ident", [IDROWS, 128], BF16, kind="ExternalInput")
    if any_cmp:
        dstloc_in = nc.dram_tensor("dstloc", [128, NT], BF16, kind="ExternalInput")
        iota_in = nc.dram_tensor("iota", [128, 128], BF16, kind="ExternalInput")
    if with_bias:
        bcat_in = nc.dram_tensor("bcat", [128, TCOLS], BF16, kind="ExternalInput")
        ones_in = nc.dram_tensor("ones", [128, 128], BF16, kind="ExternalInput")
    if mode == "elu":
        OCOLS = FOUT
        ODT = BF16
    else:
        OCOLS = C
        ODT = F32
    out_d = nc.dram_tensor("out", [core_rows, OCOLS], ODT, kind="ExternalOutput")
    table = nc.dram_tensor("table", [N, TCOLS], BF16)

    CH = ST * 128                 # xT chunk columns
    n_groups = (n_blocks + G - 1) // G

    with tile.TileContext(nc) as tc:
        with (
            tc.tile_pool(name="const", bufs=1) as kpool,
            tc.tile_pool(name="xchunk", bufs=x_bufs) as xpool,
            tc.tile_pool(name="stage", bufs=3) as stpool,
            tc.tile_pool(name="g", bufs=g_bufs) as gpool,
            tc.tile_pool(name="sf", bufs=2) as sfpool,
            tc.tile_pool(name="s01", bufs=max(3, pre_s01 + 3)) as spool,
            tc.tile_pool(name="mp", bufs=mp_bufs) as mppool,
            tc.tile_pool(name="epi", bufs=2) as epool,
            tc.tile_pool(name="oacc", bufs=1) as opool,
        ):
            # ---- constants ----
            wcat_sb = kpool.tile([128, KCH * TCOLS], XDT)
            nc.sync.dma_start(out=wcat_sb[:], in_=wcat[:].rearrange("p k c -> p (k c)"))
            srcoff_sb = kpool.tile([128, NT], I32)
            nc.sync.dma_start(out=srcoff_sb[:], in_=srcoff_in[:])
            aldoff_sb = kpool.tile([128, NT], I32)
            nc.sync.dma_start(out=aldoff_sb[:], in_=aldoff_in[:])
            alsoff_sb = kpool.tile([128, NT], I32)
            nc.sync.dma_start(out=alsoff_sb[:], in_=alsoff_in[:])
            if any_gath:
                s01off_sb = kpool.tile([128, NT], I32)
                nc.sync.dma_start(out=s01off_sb[:], in_=s01off_in[:])
            if any_cmp:
                dstloc_sb = kpool.tile([128, NT], BF16)
                nc.sync.dma_start(out=dstloc_sb[:], in_=dstloc_in[:])
                iota_sb = kpool.tile([128, 128], BF16)
                nc.sync.dma_start(out=iota_sb[:], in_=iota_in[:])
            if with_bias:
                bcat_sb = kpool.tile([128, TCOLS], BF16)
                nc.sync.dma_start(out=bcat_sb[:], in_=bcat_in[:])
                ones_sb = kpool.tile([128, 128], BF16)
                nc.sync.dma_start(out=ones_sb[:], in_=ones_in[:])
            neg1 = kpool.tile([128, 1], F32)
            nc.vector.memset(neg1[:], -1.0)

            obuf = opool.tile([128, n_blocks * OCOLS], ODT)

            table_flat = bass.AP(table[:].tensor, 0,
                                 [[N * TCOLS, 1], [1, N * TCOLS]])
            if any_gath:
                ident_flat = bass.AP(ident_in[:].tensor, 0,
                                     [[IDROWS * 128, 1], [1, IDROWS * 128]])

            # ---- s01 build helper ----
            s01_pre = {}

            def build_s01(b, split):
                bT = b * T_blk
                sd, sp = split
                sg = T_blk - sd - sp
                s01 = spool.tile([128, T_blk * 128], BF16, tag="s01")
                t0 = 0
                if sg:
                    nc.gpsimd.indirect_dma_start(
                        out=s01[:, 0:sg * 128], out_offset=None,
                        in_=ident_flat,
                        in_offset=bass.IndirectOffsetOnAxis(
                            ap=s01off_sb[:, bT:bT + sg], axis=1),
                    )
                    t0 = sg
                if sd:
                    nc.vector.tensor_tensor(
                        out=s01[:, t0 * 128:(t0 + sd) * 128],
                        in0=mkap(dstloc_sb[:], [[1, sd], [0, 128]],
                                 elem_offset=bT + t0),
                        in1=mkap(iota_sb[:], [[0, sd], [1, 128]]),
                        op=ALU.is_equal,
                    )
                    t0 += sd
                if sp:
                    nc.gpsimd.tensor_tensor(
                        out=s01[:, t0 * 128:(t0 + sp) * 128],
                        in0=mkap(dstloc_sb[:], [[1, sp], [0, 128]],
                                 elem_offset=bT + t0),
                        in1=mkap(iota_sb[:], [[0, sp], [1, 128]]),
                        op=ALU.is_equal,
                    )
                return s01

            # ---- dense phase: table[N, TCOLS] = x @ Wcat (+ b') ----
            with tc.tile_pool(name="dpsum", bufs=2, space="PSUM") as dppool:
                n_ch = (N + CH - 1) // CH
                pre_every = max(1, n_ch // pre_s01) if pre_s01 else 0
                cp_i = 0
                for ci in range(n_ch):
                    if pre_every and ci % pre_every == 0 and len(s01_pre) < pre_s01:
                        b = len(s01_pre)
                        s01_pre[b] = build_s01(b, pre_split)
                    c0 = ci * CH
                    ccols = min(CH, N - c0)
                    nt_ch = (ccols + 127) // 128
                    xc = xpool.tile([128, KCH * CH], XDT, tag="xc")
                    nc.sync.dma_start(
                        out=mkap(xc[:], [[CH, KCH], [1, ccols]]),
                        in_=xT[:, :, c0:c0 + ccols],
                    )
                    st = stpool.tile([128, ST * TCOLS], BF16, tag="st")
                    for pr in range((nt_ch + 1) // 2):
                        psum = dppool.tile([128, 1024], F32, tag="dp")
                        nsub = min(2, nt_ch - pr * 2)
                        for sub in range(nsub):
                            tl = pr * 2 + sub
                            col = tl * 128
                            rows = min(128, ccols - col)
                            for k in range(KCH):
                                nc.tensor.matmul(
                                    psum[:rows, sub * 512: sub * 512 + TCOLS],
                                    lhsT=xc[:, k * CH + col: k * CH + col + rows],
                                    rhs=wcat_sb[:, k * TCOLS:(k + 1) * TCOLS],
                                    start=(k == 0),
                                    stop=(not with_bias and k == KCH - 1),
                                )
                            if with_bias:
                                nc.tensor.matmul(
                                    psum[:rows, sub * 512: sub * 512 + TCOLS],
                                    lhsT=ones_sb[:, 0:rows],
                                    rhs=bcat_sb[:],
                                    start=False, stop=True,
                                )
                        dst_ap = mkap(st[:], [[TCOLS, nsub], [1, TCOLS]],
                                      elem_offset=pr * 2 * TCOLS)
                        src_ap = mkap(psum[:], [[512, nsub], [1, TCOLS]])
                        # Pool/gpsimd cannot read PSUM: copies go to Act/DVE only
                        eng = copy_rr[cp_i % len(copy_rr)]
                        cp_i += 1
                        if eng == "act":
                            nc.scalar.copy(out=dst_ap, in_=src_ap)
                        else:
                            nc.vector.tensor_copy(out=dst_ap, in_=src_ap)
                    # flush: full 128-row tiles in one strided DMA, tail separately
                    n_full = ccols // 128
                    if n_full:
                        nc.sync.dma_start(
                            out=table[c0:c0 + n_full * 128, :]
                            .rearrange("(a p) c -> p a c", p=128),
                            in_=mkap(st[:], [[TCOLS, n_full], [1, TCOLS]]),
                        )
                    rem = ccols - n_full * 128
                    if rem:
                        nc.sync.dma_start(
                            out=table[c0 + n_full * 128: c0 + ccols, :],
                            in_=st[:rem, n_full * TCOLS:(n_full + 1) * TCOLS],
                        )

            # ---- edge phase: groups of G blocks ----
            with tc.tile_pool(name="upsum", bufs=2, space="PSUM") as uppool:
                def group_front(gi):
                    b0 = gi * G
                    gb = min(G, n_blocks - b0)
                    gT = gb * T_blk
                    o0 = b0 * T_blk
                    # per-edge attention logits: al_src[src] + al_dst[dst],
                    # summed in f32 to keep logit precision
                    ga = sfpool.tile([128, G * T_blk * H], BF16, tag="ga")
                    nc.gpsimd.indirect_dma_start(
                        out=ga[:, 0:gT * H], out_offset=None, in_=table_flat,
                        in_offset=bass.IndirectOffsetOnAxis(
                            ap=aldoff_sb[:, o0:o0 + gT], axis=1),
                    )
                    gs = sfpool.tile([128, G * T_blk * H], BF16, tag="gs")
                    nc.gpsimd.indirect_dma_start(
                        out=gs[:, 0:gT * H], out_offset=None, in_=table_flat,
                        in_offset=bass.IndirectOffsetOnAxis(
                            ap=alsoff_sb[:, o0:o0 + gT], axis=1),
                    )
                    sf = sfpool.tile([128, G * T_blk * H], F32, tag="sf")
                    nc.vector.tensor_tensor(
                        out=sf[:, 0:gT * H], in0=ga[:, 0:gT * H],
                        in1=gs[:, 0:gT * H], op=ALU.add)
                    lr = sfpool.tile([128, G * T_blk * H], F32, tag="lr")
                    nc.vector.scalar_tensor_tensor(
                        out=lr[:, 0:gT * H], in0=sf[:, 0:gT * H], scalar=0.2,
                        in1=sf[:, 0:gT * H], op0=ALU.mult, op1=ALU.max)
                    up = uppool.tile([128, G * 512], F32, tag="up")
                    gtiles, mtiles, stiles = [], [], []
                    for q in range(gb):
                        b = b0 + q
                        bT = b * T_blk
                        # payload gather: h rows (c-major) of all edge sources
                        g_blk = gpool.tile([128, T_blk * FOUT], BF16, tag="g")
                        nc.gpsimd.indirect_dma_start(
                            out=g_blk[:], out_offset=None, in_=table_flat,
                            in_offset=bass.IndirectOffsetOnAxis(
                                ap=srcoff_sb[:, bT:bT + T_blk], axis=1),
                        )
                        gtiles.append(g_blk)
                        # selection matrix s01[j, (t,d)] = (dstloc[j,t] == d)
                        if b in s01_pre:
                            s01 = s01_pre.pop(b)
                        else:
                            s01 = build_s01(b, s01_split)
                        stiles.append(s01)
                        # mpee[:, t*GC:] = [ee*h (c-major) | ee]
                        mpee = mppool.tile([128, T_blk * GC], BF16, tag="mpee")
                        nc.scalar.activation(
                            mkap(mpee[:], [[GC, T_blk], [1, H]], elem_offset=FOUT),
                            lr[:, q * T_blk * H:(q + 1) * T_blk * H],
                            ACTF.Exp)
                        n_mpv = T_blk - mp_pool
                        nc.vector.tensor_tensor(
                            out=mkap(mpee[:], [[GC, n_mpv], [1, FOUT]]),
                            in0=g_blk[:, 0:n_mpv * FOUT],
                            in1=mkap(mpee[:], [[GC, n_mpv], [0, C], [1, H]],
                                     elem_offset=FOUT),
                            op=ALU.mult,
                        )
                        if mp_pool:
                            nc.gpsimd.tensor_tensor(
                                out=mkap(mpee[:], [[GC, mp_pool], [1, FOUT]],
                                         elem_offset=n_mpv * GC),
                                in0=g_blk[:, n_mpv * FOUT:T_blk * FOUT],
                                in1=mkap(mpee[:], [[GC, mp_pool], [0, C], [1, H]],
                                         elem_offset=n_mpv * GC + FOUT),
                                op=ALU.mult,
                            )
                        mtiles.append(mpee)
                    for q in range(gb):
                        s01 = stiles[q]
                        mpee = mtiles[q]
                        for t in range(T_blk):
                            nc.tensor.matmul(
                                up[:, q * 512: q * 512 + GC],
                                lhsT=s01[:, t * 128:(t + 1) * 128],
                                rhs=mpee[:, t * GC:(t + 1) * GC],
                                start=(t == 0),
                                stop=(t == T_blk - 1),
                            )
                    return up

                def group_epi(gi, up):
                    b0 = gi * G
                    gb = min(G, n_blocks - b0)
                    rec = epool.tile([128, G * H], F32, tag="rec")
                    nc.vector.reciprocal(
                        out=rec[:, 0:gb * H],
                        in_=mkap(up[:], [[512, gb], [1, H]], elem_offset=FOUT))
                    # NOTE: up lives in PSUM; only DVE/Act may touch PSUM
                    u = epool.tile([128, G * FOUT], F32, tag="u")
                    nc.vector.tensor_tensor(
                        out=u[:, 0:gb * FOUT],
                        in0=mkap(up[:], [[512, gb], [1, FOUT]]),
                        in1=mkap(rec[:], [[H, gb], [0, C], [1, H]]),
                        op=ALU.mult)
                    if mode == "elu":
                        nr = epool.tile([128, G * FOUT], F32, tag="nr")
                        nc.scalar.activation(nr[:, 0:gb * FOUT], u[:, 0:gb * FOUT],
                                             ACTF.Relu, scale=-1.0)
                        ex = epool.tile([128, G * FOUT], F32, tag="ex")
                        nc.scalar.activation(ex[:, 0:gb * FOUT], nr[:, 0:gb * FOUT],
                                             ACTF.Exp, scale=-1.0)
                        sm = epool.tile([128, G * FOUT], F32, tag="sm")
                        sm_e = nc.vector if sm_eng == "dve" else nc.gpsimd
                        sm_e.scalar_tensor_tensor(
                            out=sm[:, 0:gb * FOUT], in0=u[:, 0:gb * FOUT],
                            scalar=0.0, in1=ex[:, 0:gb * FOUT],
                            op0=ALU.max, op1=ALU.add)
                        nc.scalar.activation(
                            obuf[:, b0 * OCOLS:(b0 + gb) * OCOLS],
                            sm[:, 0:gb * FOUT],
                            ACTF.Identity, bias=neg1[:])
                    else:
                        # mean over heads (c-major: pairwise over inner h)
                        m4 = epool.tile([128, G * C * 4], F32, tag="m4")
                        nc.vector.tensor_tensor(
                            out=m4[:, 0:gb * C * 4],
                            in0=mkap(u[:], [[FOUT, gb], [H, C], [1, 4]]),
                            in1=mkap(u[:], [[FOUT, gb], [H, C], [1, 4]],
                                     elem_offset=4),
                            op=ALU.add)
                        m2 = epool.tile([128, G * C * 2], F32, tag="m2")
                        nc.vector.tensor_tensor(
                            out=m2[:, 0:gb * C * 2],
                            in0=mkap(m4[:], [[C * 4, gb], [4, C], [1, 2]]),
                            in1=mkap(m4[:], [[C * 4, gb], [4, C], [1, 2]],
                                     elem_offset=2),
                            op=ALU.add)
                        m1 = epool.tile([128, G * C], F32, tag="m1")
                        nc.vector.tensor_tensor(
                            out=m1[:, 0:gb * C],
                            in0=mkap(m2[:], [[C * 2, gb], [2, C]]),
                            in1=mkap(m2[:], [[C * 2, gb], [2, C]], elem_offset=1),
                            op=ALU.add)
                        mx = epool.tile([128, G], F32, tag="mx")
                        nc.vector.reduce_max(
                            out=mx[:, 0:gb],
                            in_=mkap(m1[:], [[C, gb], [1, C]]),
                            axis=mybir.AxisListType.X)
                        nmx = epool.tile([128, G], F32, tag="nmx")
                        nc.vector.tensor_scalar_mul(out=nmx[:, 0:gb],
                                                    in0=mx[:, 0:gb],
                                                    scalar1=-1.0 / H)
                        exs = epool.tile([128, C], F32, tag="exs")
                        sms = epool.tile([128, G], F32, tag="sms")
                        for q in range(gb):
                            nc.scalar.activation(
                                exs[:], m1[:, q * C:(q + 1) * C], ACTF.Exp,
                                scale=1.0 / H, bias=nmx[:, q:q + 1],
                                accum_out=sms[:, q:q + 1])
                        lg = epool.tile([128, G], F32, tag="lg")
                        nc.scalar.activation(lg[:, 0:gb], sms[:, 0:gb], ACTF.Ln)
                        nb = epool.tile([128, G], F32, tag="nb")
                        nc.vector.tensor_tensor(out=nb[:, 0:gb], in0=nmx[:, 0:gb],
                                                in1=lg[:, 0:gb], op=ALU.subtract)
                        for q in range(gb):
                            nc.scalar.activation(
                                obuf[:, (b0 + q) * OCOLS:(b0 + q + 1) * OCOLS],
                                m1[:, q * C:(q + 1) * C],
                                ACTF.Identity, scale=1.0 / H, bias=nb[:, q:q + 1])

                    # flush this group's output rows (tail block separately);
                    # runs for both modes
                    nfull_g = gb - 1 if b0 + gb == n_blocks else gb
                    if nfull_g:
                        nc.sync.dma_start(
                            out=out_d[b0 * 128:(b0 + nfull_g) * 128, :]
                            .rearrange("(a p) c -> p a c", p=128),
                            in_=mkap(obuf[:], [[OCOLS, nfull_g], [1, OCOLS]],
                                     elem_offset=b0 * OCOLS),
                        )
                    if b0 + gb == n_blocks:
                        lbr = core_rows - (n_blocks - 1) * 128
                        nc.sync.dma_start(
                            out=out_d[(n_blocks - 1) * 128:core_rows, :],
                            in_=obuf[:lbr, (n_blocks - 1) * OCOLS:
                                     n_blocks * OCOLS],
                        )

                prev = None
                for gi in range(n_groups):
                    up = group_front(gi)
                    if prev is not None:
                        group_epi(gi - 1, prev)
                    prev = up
                group_epi(n_groups - 1, prev)

    return nc


# ---------------- host side ----------------

def fold_weights_cmajor(W, a_src, a_dst, H, C, in_perm=None):
    """Wcat [FIN, C*H + 2H] f32: [W.T cols c-major | Wa_src | Wa_dst].

    in_perm permutes the FIN axis (to match a c-major input layout)."""
    WT = np.asarray(W, np.float32).T.copy()           # [FIN, H*C]
    FIN = WT.shape[0]
    W3 = WT.reshape(FIN, H, C)
    Wc = np.ascontiguousarray(W3.transpose(0, 2, 1)).reshape(FIN, C * H)
    Wa_s = np.einsum('fhc,hc->fh', W3, np.asarray(a_src, np.float32))
    Wa_d = np.einsum('fhc,hc->fh', W3, np.asarray(a_dst, np.float32))
    M = np.concatenate([Wc, Wa_s, Wa_d], axis=1)
    if in_perm is not None:
        M = M[in_perm]
    return M


def pack_kdim(M):
    """[FIN, COLS] -> [128, KCH, COLS]: row k*128+p -> [p, k]."""
    FIN, COLS = M.shape
    KCH = FIN // 128
    return np.ascontiguousarray(M.reshape(KCH, 128, COLS).transpose(1, 0, 2))


def route_edges(src, dst, n_cores, core_rows, n_nodes):
    """Balanced dst-node placement + per-core edge routing (self-loops kept).

    Returns (T_blk, n_blocks, perm_pos[n_nodes], per-core dict of index
    arrays [128, NT])."""
    import heapq
    n_blocks = (core_rows + 127) // 128
    NBLK = n_cores * n_blocks
    last_blk_rows = core_rows - (n_blocks - 1) * 128
    cap0 = np.full(NBLK, 128, np.int64)
    cap0[n_blocks - 1::n_blocks] = last_blk_rows

    deg = np.bincount(dst, minlength=n_nodes).astype(np.int64)
    order = np.argsort(-deg, kind='stable')

    assign_blk = np.empty(n_nodes, np.int64)
    slot_of = np.empty(n_nodes, np.int64)
    cap = cap0.copy()
    fill = np.zeros(NBLK, np.int64)
    heap = [(0, b) for b in range(NBLK)]
    heapq.heapify(heap)
    for n in order:
        s, b = heapq.heappop(heap)
        assign_blk[n] = b
        slot_of[n] = fill[b]
        fill[b] += 1
        cap[b] -= 1
        if cap[b]:
            heapq.heappush(heap, (s + deg[n], b))

    core_of_blk = assign_blk // n_blocks
    lblk_of = assign_blk % n_blocks
    perm_pos = core_of_blk * core_rows + lblk_of * 128 + slot_of

    eblk = assign_blk[dst]
    cnt = np.bincount(eblk, minlength=NBLK)
    # fake edges to fill the unused slots of each core's last block, so no
    # dst slot has an empty softmax denominator (avoids inf/NaN lanes)
    n_fake = 128 - last_blk_rows
    need = cnt.copy()
    if n_fake:
        need[n_blocks - 1::n_blocks] += n_fake
    T_blk = int(np.ceil(need.max() / 128.0))
    capE = T_blk * 128

    order_e = np.lexsort((src, eblk))
    se, de = src[order_e], dst[order_e]
    blk_sorted = eblk[order_e]
    starts = np.zeros(NBLK + 1, np.int64)
    np.cumsum(cnt, out=starts[1:])
    sidx = np.zeros((NBLK, capE), np.int32)
    didx = np.zeros((NBLK, capE), np.int32)
    dloc = np.full((NBLK, capE), -1, np.int32)
    pos_in_blk = np.arange(len(se)) - starts[blk_sorted]
    sidx[blk_sorted, pos_in_blk] = se
    didx[blk_sorted, pos_in_blk] = de
    dloc[blk_sorted, pos_in_blk] = slot_of[de]
    if n_fake:
        lastb = np.arange(n_blocks - 1, NBLK, n_blocks)
        for b in lastb:
            e0 = cnt[b]
            dloc[b, e0:e0 + n_fake] = np.arange(last_blk_rows, 128)
            # didx stays 0: al_dst read from row 0, harmless

    out = []
    for c in range(n_cores):
        lo, hi = c * n_blocks, (c + 1) * n_blocks
        # [n_blocks, T_blk, 128] -> [128, n_blocks*T_blk]
        def core_arr(a):
            return np.ascontiguousarray(a[lo:hi].reshape(n_blocks * T_blk, 128).T)
        out.append({"sidx": core_arr(sidx), "didx": core_arr(didx),
                    "dloc": core_arr(dloc)})
    return T_blk, n_blocks, perm_pos, out


def index_inputs(routed_core, tcols, fout, hh):
    """Per-core gather-offset arrays from routed sidx/didx/dloc."""
    sidx = routed_core["sidx"].astype(np.int64)
    didx = routed_core["didx"].astype(np.int64)
    dloc = routed_core["dloc"].astype(np.int64)
    srcoff = (sidx * tcols).astype(np.int32)
    aldoff = (didx * tcols + fout + hh).astype(np.int32)
    alsoff = (sidx * tcols + fout).astype(np.int32)
    s01off = np.where(dloc < 0, 128 * 128, dloc * 128).astype(np.int32)
    dstloc = dloc.astype(np.float32).astype(bf16)
    return {"srcoff": srcoff, "aldoff": aldoff, "alsoff": alsoff,
            "s01off": s01off, "dstloc": dstloc}


MAX_WAITS = 1


def fix_excess_waits(nc):
    """Post-process BIR JSON: any instruction with >MAX_WAITS sem-waits gets
    preceding Nop instructions carrying the excess waits (same engine, in-order).
    Monkeypatches nc.to_json_bytes to return the fixed JSON."""
    raw = nc.to_json_bytes()
    d = json.loads(raw)
    n_fix = 0
    for f in d["functions"]:
        for bb in f["blocks"]:
            out = []
            for inst in bb["instructions"]:
                si = inst.get("sync_info")
                waits = (si or {}).get("on_wait") or []
                if len(waits) > MAX_WAITS:
                    extra = waits[:-MAX_WAITS]
                    keep = waits[-MAX_WAITS:]
                    for ci in range(0, len(extra), MAX_WAITS):
                        chunk = extra[ci:ci + MAX_WAITS]
                        n_fix += 1
                        out.append({
                            "debug": inst.get("debug", 0),
                            "engine": inst["engine"],
                            "ins": [],
                            "is_reset_sema": False,
                            "name": f"{inst['name']}-wfix{ci}",
                            "opcode": "EventSemaphore",
                            "outs": [],
                            "sync_info": {"on_update": [], "on_wait": chunk},
                        })
                    si["on_wait"] = keep
                out.append(inst)
            bb["instructions"] = out
    fixed = json.dumps(d).encode()
    nc.to_json_bytes = lambda: fixed
    return n_fix


# ---------------- top-level kernel ----------------

N_NODES = 50000
N_CORES = 8
CORE_ROWS = N_NODES // N_CORES
H1, C1 = 8, 32
H2, C2 = 8, 40
# layer tuning: s01 split (n_dve, n_pool) with the rest from identity-gather,
# prebuild window, engine assignments
TUNE1 = dict(s01_split=(6, 0), pre_s01=8, copy_rr=("act",), x_fp8=False)
TUNE2 = dict(s01_split=(4, 0), pre_s01=10, copy_rr=("act", "dve"), x_fp8=True)
_CACHE = {}


def _get_program(key, builder):
    if key not in _CACHE:
        nc = builder()
        fix_excess_waits(nc)
        _CACHE[key] = nc
    return _CACHE[key]


def _make_bcat(b, H, C, TCOLS, mode):
    """Per-row bias/128 for the ones-matmul (c-major): [b' | 0 | 0]."""
    b = np.asarray(b, np.float32)
    row = np.zeros(TCOLS, np.float32)
    if mode == "elu":
        row[:H * C] = b.reshape(H, C).T.ravel()
    else:
        row[:H * C] = np.tile(b, H).reshape(H, C).T.ravel()
    return np.tile((row / 128.0)[None, :], (128, 1)).astype(bf16)


def kernel(x, edge_index, W1, a_src1, a_dst1, b1, W2, a_src2, a_dst2, b2):
    from concourse.bass_utils import run_bass_kernel_spmd

    x = np.asarray(x, np.float32)
    ei = np.asarray(edge_index)
    N = N_NODES
    src = np.concatenate([ei[0], np.arange(N)]).astype(np.int64)
    dst = np.concatenate([ei[1], np.arange(N)]).astype(np.int64)
    T_blk, n_blocks, perm_pos, routed = route_edges(
        src, dst, N_CORES, CORE_ROWS, N)

    iota_rows = np.tile(np.arange(128, dtype=np.float32)[None, :],
                        (128, 1)).astype(bf16)
    ident = np.zeros((IDROWS, 128), np.float32)
    ident[:128, :128] = np.eye(128)
    ident = ident.astype(bf16)
    ones_sq = np.ones((128, 128), np.float32).astype(bf16)

    # ---- layer 1 ----
    FOUT1 = H1 * C1
    Wcat1 = fold_weights_cmajor(W1, a_src1, a_dst1, H1, C1)
    TC1 = Wcat1.shape[1]
    wb1 = bool(np.any(np.asarray(b1, np.float32) != 0))
    nc1 = _get_program(("l1", T_blk, n_blocks, wb1, str(TUNE1)), lambda: build_gat_layer(
        N, 128, H1, C1, T_blk, n_blocks, CORE_ROWS, "elu", with_bias=wb1,
        **TUNE1))
    xdt1 = ml_dtypes.float8_e4m3 if TUNE1.get("x_fp8") else bf16
    com1 = {
        "xT": pack_kdim(np.ascontiguousarray(x.T)).astype(xdt1),
        "wcat": pack_kdim(Wcat1).astype(xdt1),
        "iota": iota_rows, "ident": ident,
    }
    if wb1:
        com1["bcat"] = _make_bcat(b1, H1, C1, TC1, "elu")
        com1["ones"] = ones_sq
    in_maps1 = [dict(com1, **index_inputs(routed[c], TC1, FOUT1, H1))
                for c in range(N_CORES)]
    in_maps1 = [_filter_inputs(nc1, m) for m in in_maps1]
    res1 = run_bass_kernel_spmd(nc1, in_maps1, list(range(N_CORES)))
    h1p = np.concatenate([np.asarray(res1.results[c]["out"])
                          for c in range(N_CORES)], axis=0)
    h1 = np.asarray(h1p, np.float32)[perm_pos]      # node order, cols (c1,h1)

    # ---- layer 2 ----
    FOUT2 = H2 * C2
    FIN2 = H1 * C1
    # h1 columns are c-major: our column j=c1*H1+h1 is original feature
    # f=h1*C1+c1, so permute Wcat2's input-feature rows to match
    row_perm = (np.arange(FIN2).reshape(H1, C1).T).ravel()
    Wcat2 = fold_weights_cmajor(W2, a_src2, a_dst2, H2, C2)[row_perm]
    TC2 = Wcat2.shape[1]
    wb2 = bool(np.any(np.asarray(b2, np.float32) != 0))
    nc2 = _get_program(("l2", T_blk, n_blocks, wb2, str(TUNE2)), lambda: build_gat_layer(
        N, 256, H2, C2, T_blk, n_blocks, CORE_ROWS, "mean_lsm", with_bias=wb2,
        **TUNE2))
    xdt2 = ml_dtypes.float8_e4m3 if TUNE2.get("x_fp8") else bf16
    com2 = {
        "xT": pack_kdim(np.ascontiguousarray(h1.T)).astype(xdt2),
        "wcat": pack_kdim(Wcat2).astype(xdt2),
        "iota": iota_rows, "ident": ident,
    }
    if wb2:
        com2["bcat"] = _make_bcat(b2, H2, C2, TC2, "mean_lsm")
        com2["ones"] = ones_sq
    in_maps2 = [dict(com2, **index_inputs(routed[c], TC2, FOUT2, H2))
                for c in range(N_CORES)]
    in_maps2 = [_filter_inputs(nc2, m) for m in in_maps2]
    res2 = run_bass_kernel_spmd(nc2, in_maps2, list(range(N_CORES)))
    outp = np.concatenate([np.asarray(res2.results[c]["out"])
                           for c in range(N_CORES)], axis=0)
    return np.asarray(outp, np.float32)[perm_pos]


def _filter_inputs(nc, m):
    import concourse.mybir as mb
    names = {a.memorylocations[0].name for a in nc.m.functions[0].allocations
             if isinstance(a, mb.MemoryLocationSet) and a.kind == "ExternalInput"}
    return {k: v for k, v in m.items() if k in names}


# revision 47
# speedup vs baseline: 2.5376x; 1.0415x over previous
"""GAT (2-layer, PyG-style) forward on 8 TRN2 NeuronCores.

Sharding: dst-node blocks across cores (host-permuted for per-block edge-count
balance); per-core edge lists routed by dst block on host; self-loops folded
into the edge list. Per dst block of 128 nodes:
  - one batched indirect DMA gathers the h-rows of all T_blk*128 edge sources
    (table rows are c-major per head so the alpha*h product runs in DVE 2x),
  - two tiny indirect DMAs fetch al_dst[dst] and accumulate al_src[src]
    (compute_op=add) per edge,
  - the 0/1 dst-slot selection matrix s01 is produced by gathering rows of a
    small identity table and/or is_equal compares (tunable DVE/Pool/DMA split),
  - segment softmax numerator+denominator come from one PE matmul per edge
    tile accumulating [alpha*h | alpha] into PSUM,
  - epilogues (div by denom + ELU / head-mean + log-softmax) run batched over
    groups of 4 blocks to amortize per-instruction engine overheads.
Dense phases (x @ Wcat -> node table in DRAM) are replicated per core; PSUM
-> SBUF bf16 conversion copies round-robin across Act/DVE/Pool."""
import sys
if '/opt/trn_rl_repo' not in sys.path:
    sys.path.insert(0, '/opt/trn_rl_repo')
import json
import numpy as np
import ml_dtypes

import concourse.bass as bass
import concourse.mybir as mybir
import concourse.tile as tile

bf16 = ml_dtypes.bfloat16
F32 = mybir.dt.float32
BF16 = mybir.dt.bfloat16
I32 = mybir.dt.int32
ALU = mybir.AluOpType
ACTF = mybir.ActivationFunctionType

IDROWS = 130                    # identity table rows (128 one-hot + 2 zero)


def mkap(ap, dims, elem_offset=0):
    """AP with explicit [step, count] free dims (elements) after the partition dim."""
    return bass.AP(ap.tensor, ap.offset + elem_offset,
                   [list(ap.ap[0])] + [list(d) for d in dims])


def build_gat_layer(N, FIN, H, C, T_blk, n_blocks, core_rows, mode,
                    with_bias=False, s01_split=(0, 0), G=4, ST=8,
                    pre_s01=0, pre_split=None, copy_rr=("act", "dve"),
                    u_eng="dve", sm_eng="dve", lr_eng="dve",
                    g_bufs=3, x_bufs=3, mp_bufs=3, mp_pool=0, x_fp8=False):
    """mode: 'elu' (layer 1) or 'mean_lsm' (layer 2).

    s01_split = (n_dve, n_pool): per block, the first T-n_dve-n_pool tiles of
    the selection matrix come from the identity-gather, then n_dve tiles via
    is_equal on DVE, then n_pool on Pool. The first pre_s01 blocks' s01 are
    built early (interleaved into the dense phase) with pre_split."""
    FOUT = H * C
    TCOLS = FOUT                  # table row: h (c-major per head)
    GC = FOUT + H                 # matmul rhs per tile: [mp | ee]
    NT = n_blocks * T_blk
    KCH = FIN // 128
    n_dve, n_pool = s01_split
    n_gath = T_blk - n_dve - n_pool
    assert n_gath >= 0
    if pre_split is None:
        pre_split = (T_blk, 0)
    p_dve, p_pool = pre_split
    p_gath = T_blk - p_dve - p_pool
    assert p_gath >= 0
    # neuronx-cc ISA: Pool supports only add/mult/copy — no is_equal, no
    # scalar_tensor_tensor, and no PSUM access
    assert n_pool == 0 and p_pool == 0
    assert u_eng == "dve" and sm_eng == "dve" and lr_eng == "dve"
    assert all(e in ("act", "dve") for e in copy_rr)
    any_gath = (n_gath and pre_s01 < n_blocks) or (p_gath and pre_s01)
    any_cmp = n_dve or n_pool or (pre_s01 and (p_dve or p_pool))

    # Batched indirect gathers emit up to T_blk*128 (~2.2k) descriptors per
    # instruction; the SWDGE ring (SBUF, 16 bytes/desc) must hold them. The
    # default 16 KiB scratch (1024 descs) silently overflows on hardware and
    # sprays descriptor bytes over adjacent DRAM (observed as NaN stripes in
    # the node table).
    nc = bass.Bass("TRN2", target_bir_lowering=False, debug=False, num_devices=8,
                   dynamic_dma_scratch_size=48 * 1024)

    XDT = mybir.dt.float8e4 if x_fp8 else BF16
    xT = nc.dram_tensor("xT", [128, KCH, N], XDT, kind="ExternalInput")
    wcat = nc.dram_tensor("wcat", [128, KCH, TCOLS], XDT, kind="ExternalInput")
    srcoff_in = nc.dram_tensor("srcoff", [128, NT], I32, kind="ExternalInput")
    lrpe_in = nc.dram_tensor("lrpe", [128, NT * H], BF16, kind="ExternalInput")
    if any_gath:
        s01off_in = nc.dram_tensor("s01off", [128, NT], I32, kind="ExternalInput")
        ident_in = nc.dram_tensor("ident", [IDROWS, 128], BF16, kind="ExternalInput")
    if any_cmp:
        dstloc_in = nc.dram_tensor("dstloc", [128, NT], BF16, kind="ExternalInput")
        iota_in = nc.dram_tensor("iota", [128, 128], BF16, kind="ExternalInput")
    if with_bias:
        bcat_in = nc.dram_tensor("bcat", [128, TCOLS], BF16, kind="ExternalInput")
        ones_in = nc.dram_tensor("ones", [128, 128], BF16, kind="ExternalInput")
    if mode == "elu":
        OCOLS = FOUT
        ODT = BF16
    else:
        OCOLS = C
        ODT = F32
    out_d = nc.dram_tensor("out", [core_rows, OCOLS], ODT, kind="ExternalOutput")
    table = nc.dram_tensor("table", [N, TCOLS], BF16)

    CH = ST * 128                 # xT chunk columns
    n_groups = (n_blocks + G - 1) // G

    with tile.TileContext(nc) as tc:
        with (
            tc.tile_pool(name="const", bufs=1) as kpool,
            tc.tile_pool(name="xchunk", bufs=x_bufs) as xpool,
            tc.tile_pool(name="stage", bufs=3) as stpool,
            tc.tile_pool(name="g", bufs=g_bufs) as gpool,
            tc.tile_pool(name="sf", bufs=2) as sfpool,
            tc.tile_pool(name="s01", bufs=max(3, pre_s01 + 3)) as spool,
            tc.tile_pool(name="mp", bufs=mp_bufs) as mppool,
            tc.tile_pool(name="epi", bufs=2) as epool,
            tc.tile_pool(name="oacc", bufs=2) as opool,
        ):
            # ---- constants ----
            wcat_sb = kpool.tile([128, KCH * TCOLS], XDT)
            nc.sync.dma_start(out=wcat_sb[:], in_=wcat[:].rearrange("p k c -> p (k c)"))
            srcoff_sb = kpool.tile([128, NT], I32)
            nc.sync.dma_start(out=srcoff_sb[:], in_=srcoff_in[:])
            if any_gath:
                s01off_sb = kpool.tile([128, NT], I32)
                nc.sync.dma_start(out=s01off_sb[:], in_=s01off_in[:])
            if any_cmp:
                dstloc_sb = kpool.tile([128, NT], BF16)
                nc.sync.dma_start(out=dstloc_sb[:], in_=dstloc_in[:])
                iota_sb = kpool.tile([128, 128], BF16)
                nc.sync.dma_start(out=iota_sb[:], in_=iota_in[:])
            if with_bias:
                bcat_sb = kpool.tile([128, TCOLS], BF16)
                nc.sync.dma_start(out=bcat_sb[:], in_=bcat_in[:])
                ones_sb = kpool.tile([128, 128], BF16)
                nc.sync.dma_start(out=ones_sb[:], in_=ones_in[:])
            neg1 = kpool.tile([128, 1], F32)
            nc.vector.memset(neg1[:], -1.0)


            table_flat = bass.AP(table[:].tensor, 0,
                                 [[N * TCOLS, 1], [1, N * TCOLS]])
            if any_gath:
                ident_flat = bass.AP(ident_in[:].tensor, 0,
                                     [[IDROWS * 128, 1], [1, IDROWS * 128]])

            # ---- s01 build helper ----
            s01_pre = {}

            def build_s01(b, split):
                bT = b * T_blk
                sd, sp = split
                sg = T_blk - sd - sp
                s01 = spool.tile([128, T_blk * 128], BF16, tag="s01")
                t0 = 0
                if sg:
                    nc.gpsimd.indirect_dma_start(
                        out=s01[:, 0:sg * 128], out_offset=None,
                        in_=ident_flat,
                        in_offset=bass.IndirectOffsetOnAxis(
                            ap=s01off_sb[:, bT:bT + sg], axis=1),
                    )
                    t0 = sg
                if sd:
                    nc.vector.tensor_tensor(
                        out=s01[:, t0 * 128:(t0 + sd) * 128],
                        in0=mkap(dstloc_sb[:], [[1, sd], [0, 128]],
                                 elem_offset=bT + t0),
                        in1=mkap(iota_sb[:], [[0, sd], [1, 128]]),
                        op=ALU.is_equal,
                    )
                    t0 += sd
                if sp:
                    nc.gpsimd.tensor_tensor(
                        out=s01[:, t0 * 128:(t0 + sp) * 128],
                        in0=mkap(dstloc_sb[:], [[1, sp], [0, 128]],
                                 elem_offset=bT + t0),
                        in1=mkap(iota_sb[:], [[0, sp], [1, 128]]),
                        op=ALU.is_equal,
                    )
                return s01

            # ---- dense phase: table[N, TCOLS] = x @ Wcat (+ b') ----
            with tc.tile_pool(name="dpsum", bufs=2, space="PSUM") as dppool:
                n_ch = (N + CH - 1) // CH
                pre_every = max(1, n_ch // pre_s01) if pre_s01 else 0
                cp_i = 0
                for ci in range(n_ch):
                    if pre_every and ci % pre_every == 0 and len(s01_pre) < pre_s01:
                        b = len(s01_pre)
                        s01_pre[b] = build_s01(b, pre_split)
                    c0 = ci * CH
                    ccols = min(CH, N - c0)
                    nt_ch = (ccols + 127) // 128
                    xc = xpool.tile([128, KCH * CH], XDT, tag="xc")
                    nc.sync.dma_start(
                        out=mkap(xc[:], [[CH, KCH], [1, ccols]]),
                        in_=xT[:, :, c0:c0 + ccols],
                    )
                    st = stpool.tile([128, ST * TCOLS], BF16, tag="st")
                    for pr in range((nt_ch + 1) // 2):
                        psum = dppool.tile([128, 1024], F32, tag="dp")
                        nsub = min(2, nt_ch - pr * 2)
                        for sub in range(nsub):
                            tl = pr * 2 + sub
                            col = tl * 128
                            rows = min(128, ccols - col)
                            for k in range(KCH):
                                nc.tensor.matmul(
                                    psum[:rows, sub * 512: sub * 512 + TCOLS],
                                    lhsT=xc[:, k * CH + col: k * CH + col + rows],
                                    rhs=wcat_sb[:, k * TCOLS:(k + 1) * TCOLS],
                                    start=(k == 0),
                                    stop=(not with_bias and k == KCH - 1),
                                )
                            if with_bias:
                                nc.tensor.matmul(
                                    psum[:rows, sub * 512: sub * 512 + TCOLS],
                                    lhsT=ones_sb[:, 0:rows],
                                    rhs=bcat_sb[:],
                                    start=False, stop=True,
                                )
                        min_rows = min(128, ccols - (pr * 2 + nsub - 1) * 128)
                        if min_rows == 128:
                            cps = [(128, mkap(st[:], [[TCOLS, nsub], [1, TCOLS]],
                                              elem_offset=pr * 2 * TCOLS),
                                    mkap(psum[:], [[512, nsub], [1, TCOLS]]))]
                        else:
                            cps = []
                            for sub in range(nsub):
                                tl = pr * 2 + sub
                                r = min(128, ccols - tl * 128)
                                cps.append((r,
                                            mkap(st[:], [[1, TCOLS]],
                                                 elem_offset=tl * TCOLS),
                                            mkap(psum[:], [[1, TCOLS]],
                                                 elem_offset=sub * 512)))
                        # Pool/gpsimd cannot read PSUM: copies go to Act/DVE only
                        for r, dst_ap, src_ap in cps:
                            dst_ap = bass.AP(dst_ap.tensor, dst_ap.offset,
                                             [[dst_ap.ap[0][0], r]]
                                             + [list(x) for x in dst_ap.ap[1:]])
                            src_ap = bass.AP(src_ap.tensor, src_ap.offset,
                                             [[src_ap.ap[0][0], r]]
                                             + [list(x) for x in src_ap.ap[1:]])
                            eng = copy_rr[cp_i % len(copy_rr)]
                            cp_i += 1
                            if eng == "act":
                                nc.scalar.copy(out=dst_ap, in_=src_ap)
                            else:
                                nc.vector.tensor_copy(out=dst_ap, in_=src_ap)
                    # flush: full 128-row tiles in one strided DMA, tail separately
                    n_full = ccols // 128
                    if n_full:
                        nc.sync.dma_start(
                            out=table[c0:c0 + n_full * 128, :]
                            .rearrange("(a p) c -> p a c", p=128),
                            in_=mkap(st[:], [[TCOLS, n_full], [1, TCOLS]]),
                        )
                    rem = ccols - n_full * 128
                    if rem:
                        nc.sync.dma_start(
                            out=table[c0 + n_full * 128: c0 + ccols, :],
                            in_=st[:rem, n_full * TCOLS:(n_full + 1) * TCOLS],
                        )

            # ---- edge phase: groups of G blocks ----
            with tc.tile_pool(name="upsum", bufs=2, space="PSUM") as uppool:
                def group_front(gi):
                    b0 = gi * G
                    gb = min(G, n_blocks - b0)
                    gT = gb * T_blk
                    o0 = b0 * T_blk
                    # per-edge leaky-relu attention logits (host-precomputed)
                    lr = sfpool.tile([128, G * T_blk * H], BF16, tag="lr")
                    nc.sync.dma_start(out=lr[:, 0:gT * H],
                                      in_=lrpe_in[:, o0 * H:(o0 + gT) * H])
                    up = uppool.tile([128, G * 512], F32, tag="up")
                    gtiles, mtiles, stiles = [], [], []
                    for q in range(gb):
                        b = b0 + q
                        bT = b * T_blk
                        # payload gather: h rows (c-major) of all edge sources
                        g_blk = gpool.tile([128, T_blk * FOUT], BF16, tag="g")
                        nc.gpsimd.indirect_dma_start(
                            out=g_blk[:], out_offset=None, in_=table_flat,
                            in_offset=bass.IndirectOffsetOnAxis(
                                ap=srcoff_sb[:, bT:bT + T_blk], axis=1),
                        )
                        gtiles.append(g_blk)
                        # selection matrix s01[j, (t,d)] = (dstloc[j,t] == d)
                        if b in s01_pre:
                            s01 = s01_pre.pop(b)
                        else:
                            s01 = build_s01(b, s01_split)
                        stiles.append(s01)
                        # mpee[:, t*GC:] = [ee*h (c-major) | ee]
                        mpee = mppool.tile([128, T_blk * GC], BF16, tag="mpee")
                        nc.scalar.activation(
                            mkap(mpee[:], [[GC, T_blk], [1, H]], elem_offset=FOUT),
                            lr[:, q * T_blk * H:(q + 1) * T_blk * H],
                            ACTF.Exp)
                        n_mpv = T_blk - mp_pool
                        nc.vector.tensor_tensor(
                            out=mkap(mpee[:], [[GC, n_mpv], [1, FOUT]]),
                            in0=g_blk[:, 0:n_mpv * FOUT],
                            in1=mkap(mpee[:], [[GC, n_mpv], [0, C], [1, H]],
                                     elem_offset=FOUT),
                            op=ALU.mult,
                        )
                        if mp_pool:
                            nc.gpsimd.tensor_tensor(
                                out=mkap(mpee[:], [[GC, mp_pool], [1, FOUT]],
                                         elem_offset=n_mpv * GC),
                                in0=g_blk[:, n_mpv * FOUT:T_blk * FOUT],
                                in1=mkap(mpee[:], [[GC, mp_pool], [0, C], [1, H]],
                                         elem_offset=n_mpv * GC + FOUT),
                                op=ALU.mult,
                            )
                        mtiles.append(mpee)
                    for q in range(gb):
                        s01 = stiles[q]
                        mpee = mtiles[q]
                        for t in range(T_blk):
                            nc.tensor.matmul(
                                up[:, q * 512: q * 512 + GC],
                                lhsT=s01[:, t * 128:(t + 1) * 128],
                                rhs=mpee[:, t * GC:(t + 1) * GC],
                                start=(t == 0),
                                stop=(t == T_blk - 1),
                            )
                    return up

                def group_epi(gi, up):
                    b0 = gi * G
                    gb = min(G, n_blocks - b0)
                    obuf = opool.tile([128, G * OCOLS], ODT, tag="ob")
                    ob0 = -b0 * OCOLS   # so obuf[:, (b0+q)*OCOLS+ob0] == local
                    rec = epool.tile([128, G * H], F32, tag="rec")
                    nc.vector.reciprocal(
                        out=rec[:, 0:gb * H],
                        in_=mkap(up[:], [[512, gb], [1, H]], elem_offset=FOUT))
                    # NOTE: up lives in PSUM; only DVE/Act may touch PSUM
                    u = epool.tile([128, G * FOUT], F32, tag="u")
                    nc.vector.tensor_tensor(
                        out=u[:, 0:gb * FOUT],
                        in0=mkap(up[:], [[512, gb], [1, FOUT]]),
                        in1=mkap(rec[:], [[H, gb], [0, C], [1, H]]),
                        op=ALU.mult)
                    if mode == "elu":
                        nr = epool.tile([128, G * FOUT], F32, tag="nr")
                        nc.scalar.activation(nr[:, 0:gb * FOUT], u[:, 0:gb * FOUT],
                                             ACTF.Relu, scale=-1.0)
                        ex = epool.tile([128, G * FOUT], F32, tag="ex")
                        nc.scalar.activation(ex[:, 0:gb * FOUT], nr[:, 0:gb * FOUT],
                                             ACTF.Exp, scale=-1.0)
                        sm = epool.tile([128, G * FOUT], F32, tag="sm")
                        sm_e = nc.vector if sm_eng == "dve" else nc.gpsimd
                        sm_e.scalar_tensor_tensor(
                            out=sm[:, 0:gb * FOUT], in0=u[:, 0:gb * FOUT],
                            scalar=0.0, in1=ex[:, 0:gb * FOUT],
                            op0=ALU.max, op1=ALU.add)
                        nc.scalar.activation(
                            obuf[:, 0:gb * OCOLS],
                            sm[:, 0:gb * FOUT],
                            ACTF.Identity, bias=neg1[:])
                    else:
                        # mean over heads (c-major: pairwise over inner h)
                        m4 = epool.tile([128, G * C * 4], F32, tag="m4")
                        nc.vector.tensor_tensor(
                            out=m4[:, 0:gb * C * 4],
                            in0=mkap(u[:], [[FOUT, gb], [H, C], [1, 4]]),
                            in1=mkap(u[:], [[FOUT, gb], [H, C], [1, 4]],
                                     elem_offset=4),
                            op=ALU.add)
                        m2 = epool.tile([128, G * C * 2], F32, tag="m2")
                        nc.vector.tensor_tensor(
                            out=m2[:, 0:gb * C * 2],
                            in0=mkap(m4[:], [[C * 4, gb], [4, C], [1, 2]]),
                            in1=mkap(m4[:], [[C * 4, gb], [4, C], [1, 2]],
                                     elem_offset=2),
                            op=ALU.add)
                        m1 = epool.tile([128, G * C], F32, tag="m1")
                        nc.vector.tensor_tensor(
                            out=m1[:, 0:gb * C],
                            in0=mkap(m2[:], [[C * 2, gb], [2, C]]),
                            in1=mkap(m2[:], [[C * 2, gb], [2, C]], elem_offset=1),
                            op=ALU.add)
                        mx = epool.tile([128, G], F32, tag="mx")
                        nc.vector.reduce_max(
                            out=mx[:, 0:gb],
                            in_=mkap(m1[:], [[C, gb], [1, C]]),
                            axis=mybir.AxisListType.X)
                        nmx = epool.tile([128, G], F32, tag="nmx")
                        nc.vector.tensor_scalar_mul(out=nmx[:, 0:gb],
                                                    in0=mx[:, 0:gb],
                                                    scalar1=-1.0 / H)
                        exs = epool.tile([128, C], F32, tag="exs")
                        sms = epool.tile([128, G], F32, tag="sms")
                        for q in range(gb):
                            nc.scalar.activation(
                                exs[:], m1[:, q * C:(q + 1) * C], ACTF.Exp,
                                scale=1.0 / H, bias=nmx[:, q:q + 1],
                                accum_out=sms[:, q:q + 1])
                        lg = epool.tile([128, G], F32, tag="lg")
                        nc.scalar.activation(lg[:, 0:gb], sms[:, 0:gb], ACTF.Ln)
                        nb = epool.tile([128, G], F32, tag="nb")
                        nc.vector.tensor_tensor(out=nb[:, 0:gb], in0=nmx[:, 0:gb],
                                                in1=lg[:, 0:gb], op=ALU.subtract)
                        for q in range(gb):
                            nc.scalar.activation(
                                obuf[:, q * OCOLS:(q + 1) * OCOLS],
                                m1[:, q * C:(q + 1) * C],
                                ACTF.Identity, scale=1.0 / H, bias=nb[:, q:q + 1])

                    # flush this group's output rows (tail block separately);
                    # runs for both modes
                    nfull_g = gb - 1 if b0 + gb == n_blocks else gb
                    if nfull_g:
                        nc.sync.dma_start(
                            out=out_d[b0 * 128:(b0 + nfull_g) * 128, :]
                            .rearrange("(a p) c -> p a c", p=128),
                            in_=mkap(obuf[:], [[OCOLS, nfull_g], [1, OCOLS]]),
                        )
                    if b0 + gb == n_blocks:
                        lbr = core_rows - (n_blocks - 1) * 128
                        nc.sync.dma_start(
                            out=out_d[(n_blocks - 1) * 128:core_rows, :],
                            in_=obuf[:lbr, (gb - 1) * OCOLS:gb * OCOLS],
                        )

                prev = None
                for gi in range(n_groups):
                    up = group_front(gi)
                    if prev is not None:
                        group_epi(gi - 1, prev)
                    prev = up
                group_epi(n_groups - 1, prev)

    return nc


# ---------------- host side ----------------

def fold_weights_cmajor(W, a_src, a_dst, H, C):
    """(Wcat [FIN, C*H] f32 with cols c-major, Wa_src [FIN,H], Wa_dst [FIN,H])."""
    WT = np.asarray(W, np.float32).T.copy()           # [FIN, H*C]
    FIN = WT.shape[0]
    W3 = WT.reshape(FIN, H, C)
    Wc = np.ascontiguousarray(W3.transpose(0, 2, 1)).reshape(FIN, C * H)
    Wa_s = np.einsum('fhc,hc->fh', W3, np.asarray(a_src, np.float32))
    Wa_d = np.einsum('fhc,hc->fh', W3, np.asarray(a_dst, np.float32))
    return Wc, Wa_s, Wa_d


def pack_kdim(M):
    """[FIN, COLS] -> [128, KCH, COLS]: row k*128+p -> [p, k]."""
    FIN, COLS = M.shape
    KCH = FIN // 128
    return np.ascontiguousarray(M.reshape(KCH, 128, COLS).transpose(1, 0, 2))


def route_edges(src, dst, n_cores, core_rows, n_nodes):
    """Balanced dst-node placement + per-core edge routing (self-loops kept).

    Returns (T_blk, n_blocks, perm_pos[n_nodes], per-core dict of index
    arrays [128, NT])."""
    import heapq
    n_blocks = (core_rows + 127) // 128
    NBLK = n_cores * n_blocks
    last_blk_rows = core_rows - (n_blocks - 1) * 128
    cap0 = np.full(NBLK, 128, np.int64)
    cap0[n_blocks - 1::n_blocks] = last_blk_rows

    deg = np.bincount(dst, minlength=n_nodes).astype(np.int64)
    order = np.argsort(-deg, kind='stable')

    assign_blk = np.empty(n_nodes, np.int64)
    slot_of = np.empty(n_nodes, np.int64)
    cap = cap0.copy()
    fill = np.zeros(NBLK, np.int64)
    heap = [(0, b) for b in range(NBLK)]
    heapq.heapify(heap)
    for n in order:
        s, b = heapq.heappop(heap)
        assign_blk[n] = b
        slot_of[n] = fill[b]
        fill[b] += 1
        cap[b] -= 1
        if cap[b]:
            heapq.heappush(heap, (s + deg[n], b))

    core_of_blk = assign_blk // n_blocks
    lblk_of = assign_blk % n_blocks
    perm_pos = core_of_blk * core_rows + lblk_of * 128 + slot_of

    eblk = assign_blk[dst]
    cnt = np.bincount(eblk, minlength=NBLK)
    # fake edges to fill the unused slots of each core's last block, so no
    # dst slot has an empty softmax denominator (avoids inf/NaN lanes)
    n_fake = 128 - last_blk_rows
    need = cnt.copy()
    if n_fake:
        need[n_blocks - 1::n_blocks] += n_fake
    T_blk = int(np.ceil(need.max() / 128.0))
    capE = T_blk * 128

    order_e = np.lexsort((src, eblk))
    se, de = src[order_e], dst[order_e]
    blk_sorted = eblk[order_e]
    starts = np.zeros(NBLK + 1, np.int64)
    np.cumsum(cnt, out=starts[1:])
    sidx = np.zeros((NBLK, capE), np.int32)
    didx = np.zeros((NBLK, capE), np.int32)
    dloc = np.full((NBLK, capE), -1, np.int32)
    pos_in_blk = np.arange(len(se)) - starts[blk_sorted]
    sidx[blk_sorted, pos_in_blk] = se
    didx[blk_sorted, pos_in_blk] = de
    dloc[blk_sorted, pos_in_blk] = slot_of[de]
    if n_fake:
        lastb = np.arange(n_blocks - 1, NBLK, n_blocks)
        for b in lastb:
            e0 = cnt[b]
            dloc[b, e0:e0 + n_fake] = np.arange(last_blk_rows, 128)
            # didx stays 0: al_dst read from row 0, harmless

    out = []
    for c in range(n_cores):
        lo, hi = c * n_blocks, (c + 1) * n_blocks
        # [n_blocks, T_blk, 128] -> [128, n_blocks*T_blk]
        def core_arr(a):
            return np.ascontiguousarray(a[lo:hi].reshape(n_blocks * T_blk, 128).T)
        out.append({"sidx": core_arr(sidx), "didx": core_arr(didx),
                    "dloc": core_arr(dloc)})
    return T_blk, n_blocks, perm_pos, out


def index_inputs(routed_core, tcols, als, ald):
    """Per-core gather offsets + per-edge lrelu attention logits.

    als/ald: [N, H] f32 per-node attention terms (host-computed)."""
    sidx = routed_core["sidx"].astype(np.int64)     # [128, NT]
    didx = routed_core["didx"].astype(np.int64)
    dloc = routed_core["dloc"].astype(np.int64)
    srcoff = (sidx * tcols).astype(np.int32)
    s01off = np.where(dloc < 0, 128 * 128, dloc * 128).astype(np.int32)
    dstloc = dloc.astype(np.float32).astype(bf16)
    sf = als[sidx] + ald[didx]                      # [128, NT, H]
    lrpe = np.maximum(sf, 0.2 * sf).reshape(sidx.shape[0], -1).astype(bf16)
    return {"srcoff": srcoff, "s01off": s01off, "dstloc": dstloc,
            "lrpe": np.ascontiguousarray(lrpe)}


MAX_WAITS = 1


def fix_excess_waits(nc):
    """Post-process BIR JSON: any instruction with >MAX_WAITS sem-waits gets
    preceding Nop instructions carrying the excess waits (same engine, in-order).
    Monkeypatches nc.to_json_bytes to return the fixed JSON."""
    raw = nc.to_json_bytes()
    d = json.loads(raw)
    n_fix = 0
    for f in d["functions"]:
        for bb in f["blocks"]:
            out = []
            for inst in bb["instructions"]:
                si = inst.get("sync_info")
                waits = (si or {}).get("on_wait") or []
                if len(waits) > MAX_WAITS:
                    extra = waits[:-MAX_WAITS]
                    keep = waits[-MAX_WAITS:]
                    for ci in range(0, len(extra), MAX_WAITS):
                        chunk = extra[ci:ci + MAX_WAITS]
                        n_fix += 1
                        out.append({
                            "debug": inst.get("debug", 0),
                            "engine": inst["engine"],
                            "ins": [],
                            "is_reset_sema": False,
                            "name": f"{inst['name']}-wfix{ci}",
                            "opcode": "EventSemaphore",
                            "outs": [],
                            "sync_info": {"on_update": [], "on_wait": chunk},
                        })
                    si["on_wait"] = keep
                out.append(inst)
            bb["instructions"] = out
    fixed = json.dumps(d).encode()
    nc.to_json_bytes = lambda: fixed
    return n_fix


# ---------------- top-level kernel ----------------

N_NODES = 50000
N_CORES = 8
CORE_ROWS = N_NODES // N_CORES
H1, C1 = 8, 32
H2, C2 = 8, 40
# layer tuning: s01 split (n_dve, n_pool) with the rest from identity-gather,
# prebuild window, engine assignments
TUNE1 = dict(s01_split=(6, 0), pre_s01=8, copy_rr=("act",), x_fp8=False)
TUNE2 = dict(s01_split=(4, 0), pre_s01=10, copy_rr=("act", "dve"), x_fp8=True)
_CACHE = {}
DEBUG_STASH = {}


def _get_program(key, builder):
    if key not in _CACHE:
        nc = builder()
        fix_excess_waits(nc)
        _CACHE[key] = nc
    return _CACHE[key]


def _make_bcat(b, H, C, TCOLS, mode):
    """Per-row bias/128 for the ones-matmul (c-major)."""
    b = np.asarray(b, np.float32)
    if mode == "elu":
        row = b.reshape(H, C).T.ravel()
    else:
        row = np.tile(b, H).reshape(H, C).T.ravel()
    return np.tile((row / 128.0)[None, :], (128, 1)).astype(bf16)


def kernel(x, edge_index, W1, a_src1, a_dst1, b1, W2, a_src2, a_dst2, b2):
    from concourse.bass_utils import run_bass_kernel_spmd

    x = np.asarray(x, np.float32)
    ei = np.asarray(edge_index)
    N = N_NODES
    src = np.concatenate([ei[0], np.arange(N)]).astype(np.int64)
    dst = np.concatenate([ei[1], np.arange(N)]).astype(np.int64)
    T_blk, n_blocks, perm_pos, routed = route_edges(
        src, dst, N_CORES, CORE_ROWS, N)

    iota_rows = np.tile(np.arange(128, dtype=np.float32)[None, :],
                        (128, 1)).astype(bf16)
    ident = np.zeros((IDROWS, 128), np.float32)
    ident[:128, :128] = np.eye(128)
    ident = ident.astype(bf16)
    ones_sq = np.ones((128, 128), np.float32).astype(bf16)

    # ---- layer 1 ----
    FOUT1 = H1 * C1
    Wcat1, Wa_s1, Wa_d1 = fold_weights_cmajor(W1, a_src1, a_dst1, H1, C1)
    TC1 = Wcat1.shape[1]
    als1 = x @ Wa_s1
    ald1 = x @ Wa_d1
    wb1 = bool(np.any(np.asarray(b1, np.float32) != 0))
    nc1 = _get_program(("l1", T_blk, n_blocks, wb1, str(TUNE1)), lambda: build_gat_layer(
        N, 128, H1, C1, T_blk, n_blocks, CORE_ROWS, "elu", with_bias=wb1,
        **TUNE1))
    xdt1 = ml_dtypes.float8_e4m3 if TUNE1.get("x_fp8") else bf16
    com1 = {
        "xT": pack_kdim(np.ascontiguousarray(x.T)).astype(xdt1),
        "wcat": pack_kdim(Wcat1).astype(xdt1),
        "iota": iota_rows, "ident": ident,
    }
    if wb1:
        com1["bcat"] = _make_bcat(b1, H1, C1, TC1, "elu")
        com1["ones"] = ones_sq
    in_maps1 = [dict(com1, **index_inputs(routed[c], TC1, als1, ald1))
                for c in range(N_CORES)]
    in_maps1 = [_filter_inputs(nc1, m) for m in in_maps1]
    res1 = run_bass_kernel_spmd(nc1, in_maps1, list(range(N_CORES)))
    h1p = np.concatenate([np.asarray(res1.results[c]["out"])
                          for c in range(N_CORES)], axis=0)
    h1 = np.asarray(h1p, np.float32)[perm_pos]      # node order, cols (c1,h1)
    DEBUG_STASH["h1"] = h1

    # ---- layer 2 ----
    FOUT2 = H2 * C2
    FIN2 = H1 * C1
    # h1 columns are c-major: our column j=c1*H1+h1 is original feature
    # f=h1*C1+c1, so permute Wcat2's input-feature rows to match
    row_perm = (np.arange(FIN2).reshape(H1, C1).T).ravel()
    Wcat2, Wa_s2, Wa_d2 = fold_weights_cmajor(W2, a_src2, a_dst2, H2, C2)
    Wcat2 = Wcat2[row_perm]
    TC2 = Wcat2.shape[1]
    als2 = h1 @ Wa_s2[row_perm]
    ald2 = h1 @ Wa_d2[row_perm]
    wb2 = bool(np.any(np.asarray(b2, np.float32) != 0))
    nc2 = _get_program(("l2", T_blk, n_blocks, wb2, str(TUNE2)), lambda: build_gat_layer(
        N, 256, H2, C2, T_blk, n_blocks, CORE_ROWS, "mean_lsm", with_bias=wb2,
        **TUNE2))
    xdt2 = ml_dtypes.float8_e4m3 if TUNE2.get("x_fp8") else bf16
    com2 = {
        "xT": pack_kdim(np.ascontiguousarray(h1.T)).astype(xdt2),
        "wcat": pack_kdim(Wcat2).astype(xdt2),
        "iota": iota_rows, "ident": ident,
    }
    if wb2:
        com2["bcat"] = _make_bcat(b2, H2, C2, TC2, "mean_lsm")
        com2["ones"] = ones_sq
    in_maps2 = [dict(com2, **index_inputs(routed[c], TC2, als2, ald2))
                for c in range(N_CORES)]
    in_maps2 = [_filter_inputs(nc2, m) for m in in_maps2]
    res2 = run_bass_kernel_spmd(nc2, in_maps2, list(range(N_CORES)))
    outp = np.concatenate([np.asarray(res2.results[c]["out"])
                           for c in range(N_CORES)], axis=0)
    return np.asarray(outp, np.float32)[perm_pos]


def _filter_inputs(nc, m):
    import concourse.mybir as mb
    names = {a.memorylocations[0].name for a in nc.m.functions[0].allocations
             if isinstance(a, mb.MemoryLocationSet) and a.kind == "ExternalInput"}
    return {k: v for k, v in m.items() if k in names}
